# revision 22
# baseline (speedup 1.0000x reference)
"""CTC loss kernel for Trainium2 (8 NeuronCores, data-parallel over batch).

Strategy
--------
Per core: 64 examples. The CTC forward DP runs in probability space
(4 tensor ops per time step on DVE) with states in the free dim and
(example, direction) packed into the 128 partitions: rows 0-63 run the
forward DP for t=0..255, rows 64-127 run the suffix (backward) DP in
state-reversed coordinates for t=511..256.  The two halves are spliced
at T/2:  P = sum_s alpha_255[s] * W_255[s].

Emissions E[b,t,s] = g*(y_pred[b,t,ext_b[s]] + eps) are produced on the
TensorEngine: per (example, 128-t block), PE-transpose y_pred to
(class, t), then a one-hot matmul (ypT.T @ onehot_b) gathers all 132
state emissions for 128 t steps in one instruction.  The scalar engine
copies PSUM->SBUF fusing the g scale and g*eps bias plus the f32->bf16
cast.  A DRAM round-trip reshuffles (t-part, s) per example into the
DP's (example-part, tau-major) chunk layout.

The state dim is stored MIRRORED (guards at the top) so that 3 of the
4 DP ops have 4-byte-aligned offset-0 operands and hit the DVE 2x_1P
bf16 perf mode (measured: offset-0 non-inplace ops run 2x).

Numerics: bf16 DP state, per-8-step per-example rescale to a 2^43
setpoint (max history written out, logs added back on host), pad states
get all-zero one-hot columns so they decay.  The final splice spans
~e^-180 for tail examples, far outside f32 range, so the two final
state tiles (34KB each) are DMA'd out and the splice runs on host in
f64 (the per-example log + mean were host work already).
"""

import numpy as np

B, T, C, L = 512, 512, 96, 64
BLANK = C - 1
EPS = 1e-7
S = 2 * L + 1          # 129 states
SW = 132               # padded state width (multiple of 4)
NCORES = 8
BN = B // NCORES       # 64 examples per core
TH = T // 2            # split point
RESC = 32              # rescale period
NRESC = (TH - 1) // RESC  # 7 rescales (tau = 32,64,...,224)
SETPOINT_LOG2 = 24     # rescale setpoint 2^24 (headroom for 32 unrescaled steps)
G = 60.646622          # exp(mean_loss/T) boost; keeps alpha ~O(1) per step

_BUILT = None
_LAST_EXEC_NS = None
_LAST_RES = None


def _host_metadata(y_true):
    """ext labels, skip masks, init masks, per-state classes — from y_true.

    Everything is built in natural state order (validated layout), then
    reversed along the free dim at pack time for the mirrored device layout.
    """
    y_true = np.asarray(y_true, dtype=np.int32)
    lbl_len = (y_true != -1).sum(axis=-1).astype(np.int32)
    labels = np.where(y_true != -1, y_true, 0).astype(np.int32)
    ext = np.full((B, S), BLANK, np.int32)
    ext[:, 1::2] = labels
    ext_m2 = np.pad(ext[:, :-2], ((0, 0), (2, 0)), constant_values=BLANK)
    can_skip = ((ext != BLANK) & (ext != ext_m2)).astype(np.float32)

    m2f = np.zeros((B, SW), np.float32)
    m2f[:, :S] = can_skip
    m2b = np.zeros((B, SW), np.float32)
    for u in range(2, S):
        m2b[:, u] = can_skip[:, S - 1 - u + 2]

    mif = np.zeros((B, SW), np.float32)
    mif[:, 0] = 1.0
    mif[:, 1] = 1.0
    mib = np.zeros((B, SW), np.float32)
    mib[np.arange(B), S - 1 - 2 * lbl_len] = 1.0
    mib[np.arange(B), S - 1 - (2 * lbl_len - 1)] = 1.0

    clsf = np.full((B, SW), -1, np.int32)           # -1 -> all-zero column
    clsf[:, :S] = ext
    clsb = np.full((B, SW), -1, np.int32)
    clsb[:, :S] = ext[:, ::-1]
    return m2f, m2b, mif, mib, clsf, clsb


def _build(num_cores=NCORES, t_full=T, bn=BN):
    """Build and schedule the Bass module once."""
    import concourse.bacc as bacc
    import concourse.mybir as mybir
    import concourse.tile as tile
    from contextlib import ExitStack
    from concourse.vector_clock import ScopedClock

    # this walrus build allows a single sem wait per Drain: split the
    # TileContext end-drain's waits across a chain of drains.
    def _patched_drain_and_barrier(self, tick_clock, wait_clock):
        nc = self.nc
        drain_inst = nc.sync.drain()
        wait_clock.add_sem_waits(
            drain_inst.ins, ScopedClock({None: tick_clock.global_clock})
        )
        si = drain_inst.ins.sync_info
        waits = list(si.on_wait) if si and si.on_wait else []
        if len(waits) > 1:
            si.on_wait = waits[:1]
            for w in waits[1:]:
                extra = nc.sync.drain()
                esi = extra.ins.sync_info
                if esi is None:
                    extra.ins.sync_info = mybir.SyncInfo(on_wait=[w], on_update=[])
                else:
                    esi.on_wait = (esi.on_wait or []) + [w]
        nc.all_engine_barrier()
        assert self.sems is not None
        popped = nc._tile_sem_poison_stack.pop()
        assert popped is self._sem_poison
        nc.clear_and_free_semaphores(list(self.sems.allocated().values()))
        nc.all_engine_barrier()

    tile.TileContext._drain_and_barrier = _patched_drain_and_barrier

    f32 = mybir.dt.float32
    bf16 = mybir.dt.bfloat16
    AX = mybir.AxisListType.X
    COPY = mybir.ActivationFunctionType.Copy
    MULT = mybir.AluOpType.mult

    th = t_full // 2
    nblk = t_full // 128
    chk = 32
    nchk = th // chk
    nresc = (th - 1) // RESC

    nc = bacc.Bacc("TRN2", target_bir_lowering=False, debug=False,
                   num_devices=num_cores)
    ypred = nc.dram_tensor("ypred", [bn, t_full, C], f32, kind="ExternalInput")
    m2_in = nc.dram_tensor("m2", [128, SW], bf16, kind="ExternalInput")
    mi_in = nc.dram_tensor("minit", [128, SW], bf16, kind="ExternalInput")
    oh_in = nc.dram_tensor("onehot", [C, bn * 2 * SW], bf16, kind="ExternalInput")
    id_in = nc.dram_tensor("ident", [128, 128], f32, kind="ExternalInput")
    h_out = nc.dram_tensor("hist", [128, max(nresc, 1)], f32, kind="ExternalOutput")
    a_out = nc.dram_tensor("afin", [128, SW + 2], bf16, kind="ExternalOutput")
    g_out = nc.dram_tensor("gfin", [128, SW], bf16, kind="ExternalOutput")

    with tile.TileContext(nc) as tc, ExitStack() as ctx:
        const = ctx.enter_context(tc.tile_pool(name="const", bufs=1))
        dramp = ctx.enter_context(tc.tile_pool(name="edram", bufs=1, space="DRAM"))
        ypf_pool = ctx.enter_context(tc.tile_pool(name="ypf", bufs=4))
        ebf_pool = ctx.enter_context(tc.tile_pool(name="ebf", bufs=2))
        ec_pool = ctx.enter_context(tc.tile_pool(name="ec", bufs=3))
        ytp_pool = ctx.enter_context(tc.tile_pool(name="ytp", bufs=2, space="PSUM"))
        eps_pool = ctx.enter_context(tc.tile_pool(name="eps", bufs=4, space="PSUM"))
        yts_pool = ctx.enter_context(tc.tile_pool(name="yts", bufs=2))

        m2t = const.tile([128, SW], bf16, tag="m2t")
        mit = const.tile([128, SW], bf16, tag="mit")
        oht = const.tile([C, bn * 2 * SW], bf16, tag="oht")
        identt = const.tile([128, 128], f32, tag="identt")
        alpha = const.tile([128, SW + 2], bf16, tag="alpha")
        ut = const.tile([128, SW], bf16, tag="ut")
        vt = const.tile([128, SW], bf16, tag="vt")
        wt = const.tile([128, SW], bf16, tag="wt")
        histt = const.tile([128, max(nresc, 1)], f32, tag="histt")
        sclt = const.tile([128, 1], f32, tag="sclt")

        nc.sync.dma_start(out=m2t[:, :], in_=m2_in.ap())
        nc.sync.dma_start(out=mit[:, :], in_=mi_in.ap())
        gw = 16 * 2 * SW
        for g in range(bn // 16):
            nc.sync.dma_start(out=oht[:, g * gw:(g + 1) * gw],
                              in_=oh_in.ap()[:, g * gw:(g + 1) * gw])
        nc.sync.dma_start(out=identt[:, :], in_=id_in.ap())
        nc.vector.memset(histt[:, :], 0.0)
        nc.vector.memset(alpha[:, :], 0.0)

        # ---- phase A: emissions per 128-t block via PE one-hot matmul ----
        blk_order = []
        for i in range(nblk // 2):
            blk_order += [i, nblk - 1 - i]
        edram = {}
        for k in blk_order:
            ebf_k = ebf_pool.tile([128, bn * SW], bf16)
            dirn = 0 if k < nblk // 2 else 1
            for grp in range(bn // 16):
                ypf = ypf_pool.tile([128, 16 * C], f32)
                yp3 = ypf[:, :].rearrange("p (e c) -> p e c", c=C)
                src = ypred.ap()[grp * 16:(grp + 1) * 16,
                                 k * 128:(k + 1) * 128, :]
                nc.sync.dma_start(out=yp3[:, :, :],
                                  in_=src.rearrange("e t c -> t e c"))
                for q in range(4):          # 4 examples per PSUM group
                    ytp = ytp_pool.tile([C, 512], f32)      # one bank
                    for e4 in range(4):
                        e = q * 4 + e4
                        nc.tensor.transpose(
                            ytp[:, e4 * 128:(e4 + 1) * 128],
                            ypf[:, e * C:(e + 1) * C],
                            identt[:, :])
                    yts = yts_pool.tile([C, 512], bf16)
                    nc.scalar.activation(yts[:, :], ytp[:, :], COPY,
                                         bias=0.0, scale=1.0)
                    for e4 in range(4):
                        e = q * 4 + e4
                        ex = grp * 16 + e
                        ohoff = (ex * 2 + dirn) * SW
                        epsum = eps_pool.tile([128, SW], f32)
                        nc.tensor.matmul(
                            epsum[:, :],
                            yts[:, e4 * 128:(e4 + 1) * 128],
                            oht[:, ohoff:ohoff + SW],
                            start=True, stop=True)
                        nc.scalar.activation(
                            ebf_k[:, ex * SW:(ex + 1) * SW], epsum[:, :],
                            COPY, bias=float(G * EPS), scale=float(G))
            ed = dramp.tile([128, bn * SW], bf16, tag=f"ed{k}")
            edram[k] = ed
            # (t-part, ex*SW contiguous) -> contiguous 16.9KB per partition row
            nc.gpsimd.dma_start(out=ed[:, :], in_=ebf_k[:, :])

        # ---- phase B: reshuffle DRAM -> (example|dir partition, tau) chunks ----
        ec_tiles = []
        for j in range(nchk):
            ec = ec_pool.tile([128, chk * SW], bf16)
            ec3 = ec[:, :].rearrange("p (t s) -> p t s", s=SW)
            kf = j // 4
            tl0 = (j % 4) * chk
            kb = nblk - 1 - kf
            tb0 = 127 - (j % 4) * chk
            fsrc = edram[kf][:, :].rearrange("t (e s) -> t e s", s=SW)
            bsrc = edram[kb][:, :].rearrange("t (e s) -> t e s", s=SW)
            bslice = slice(tb0, None, -1) if tb0 - chk < 0 else slice(tb0, tb0 - chk, -1)
            nc.sync.dma_start(
                out=ec3[0:64, :, :],
                in_=fsrc[tl0:tl0 + chk, :, :].rearrange("t e s -> e t s"))
            nc.gpsimd.dma_start(
                out=ec3[64:128, :, :],
                in_=bsrc[bslice, :, :].rearrange("t e s -> e t s"))
            ec_tiles.append(ec)

        # ---- phase C: the DP (mirrored state layout, guards at top) ----
        nc.vector.tensor_mul(alpha[:, 0:SW], ec_tiles[0][:, 0:SW], mit[:, :])
        nr = 0
        for tau in range(1, th):
            ec = ec_tiles[tau // chk]
            off = (tau % chk) * SW
            nc.vector.tensor_add(ut[:, :], alpha[:, 0:SW], alpha[:, 1:1 + SW])
            nc.vector.tensor_mul(vt[:, :], alpha[:, 2:2 + SW], m2t[:, :])
            nc.vector.tensor_add(wt[:, :], ut[:, :], vt[:, :])
            nc.vector.tensor_mul(alpha[:, 0:SW], wt[:, :], ec[:, off:off + SW])
            if tau % RESC == 0 and nr < nresc:
                # max over mirrored real states i in [2, 132) (incl. one pad col)
                nc.vector.reduce_max(histt[:, nr:nr + 1], alpha[:, 2:SW], axis=AX)
                nc.vector.reciprocal_approx_fast(sclt[:, :], histt[:, nr:nr + 1])
                nc.vector.tensor_scalar(alpha[:, 0:SW], alpha[:, 0:SW],
                                        sclt[:, :], float(2.0 ** SETPOINT_LOG2),
                                        MULT, MULT)
                nr += 1

        # ---- final: gamma on bwd rows, dump states (host does f64 splice) ----
        nc.vector.tensor_add(ut[:, :], alpha[:, 0:SW], alpha[:, 1:1 + SW])
        nc.vector.tensor_mul(vt[:, :], alpha[:, 2:2 + SW], m2t[:, :])
        nc.vector.tensor_add(wt[:, :], ut[:, :], vt[:, :])
        nc.sync.dma_start(out=a_out.ap(), in_=alpha[:, :])
        nc.sync.dma_start(out=g_out.ap(), in_=wt[:, :])
        nc.sync.dma_start(out=h_out.ap(), in_=histt[:, :])

    nc.compile()
    return nc


def kernel(y_true, y_pred):
    global _BUILT, _LAST_EXEC_NS, _LAST_RES
    from concourse.bass_utils import run_bass_kernel_spmd

    y_true = np.asarray(y_true)
    y_pred = np.ascontiguousarray(np.asarray(y_pred, dtype=np.float32))

    m2f, m2b, mif, mib, clsf, clsb = _host_metadata(y_true)

    if _BUILT is None:
        _BUILT = _build()
    nc = _BUILT

    import ml_dtypes
    bf = ml_dtypes.bfloat16
    ident = np.eye(128, dtype=np.float32)
    in_maps = []
    for c in range(NCORES):
        sl = slice(c * BN, (c + 1) * BN)
        # mirrored layout: reverse the free (state) dim
        m2 = np.concatenate([m2f[sl], m2b[sl]], axis=0)[:, ::-1].astype(bf)
        mi = np.concatenate([mif[sl], mib[sl]], axis=0)[:, ::-1].astype(bf)
        oh = np.zeros((C, BN * 2 * SW), bf)
        for e in range(BN):
            b = c * BN + e
            for dirn, cls in ((0, clsf[b]), (1, clsb[b])):
                colbase = (e * 2 + dirn) * SW
                rcls = cls[::-1]                    # mirrored
                for i in range(SW):
                    if rcls[i] >= 0:
                        oh[rcls[i], colbase + i] = bf(1.0)
        in_maps.append({
            "ypred": y_pred[sl],
            "m2": np.ascontiguousarray(m2),
            "minit": np.ascontiguousarray(mi),
            "onehot": oh,
            "ident": ident,
        })

    import os
    trace = os.environ.get("CTC_TRACE", "") == "1"
    res = run_bass_kernel_spmd(nc, in_maps, list(range(NCORES)), trace=trace)
    _LAST_EXEC_NS = res.exec_time_ns
    _LAST_RES = res

    losses = np.zeros(B, np.float64)
    lng = np.log(np.float64(G))
    setlog = NRESC * SETPOINT_LOG2 * np.log(2.0)
    for c in range(NCORES):
        afin = res.results[c]["afin"].astype(np.float64)   # (128, SW+2) mirrored
        gfin = res.results[c]["gfin"].astype(np.float64)   # (128, SW) mirrored
        hist = res.results[c]["hist"].astype(np.float64)
        acc = np.log(np.maximum(hist[:, :NRESC], 1e-300)).sum(axis=1)
        afs = afin[:, 0:SW][:, ::-1]             # un-mirror -> natural state order
        gfs = gfin[:, :][:, ::-1]
        af = afs[0:64, 0:S]                      # alpha_{T/2-1}[s]
        gm = gfs[64:128, 0:S][:, ::-1]           # W_{T/2-1}[s], u -> s
        P = (af * gm).sum(axis=1)
        lnP = np.log(np.maximum(P, 1e-300))
        losses[c * BN:(c + 1) * BN] = -(
            lnP + acc[:64] + acc[64:128] - 2 * setlog - T * lng)
    return np.float32(losses.mean())



# revision 23
# speedup vs baseline: 1.0047x; 1.0047x over previous
"""CTC loss kernel for Trainium2 (8 NeuronCores, data-parallel over batch).

Strategy
--------
Per core: 64 examples. The CTC forward DP runs in probability space
(4 tensor ops per time step on DVE) with states in the free dim and
(example, direction) packed into the 128 partitions: rows 0-63 run the
forward DP for t=0..255, rows 64-127 run the suffix (backward) DP in
state-reversed coordinates for t=511..256.  The two halves are spliced
at T/2:  P = sum_s alpha_255[s] * W_255[s].

Emissions E[b,t,s] = g*(y_pred[b,t,ext_b[s]] + eps) are produced on the
TensorEngine: per (example, 128-t block), PE-transpose y_pred to
(class, t), then a one-hot matmul (ypT.T @ onehot_b) gathers all 132
state emissions for 128 t steps in one instruction.  The scalar engine
copies PSUM->SBUF fusing the g scale and g*eps bias plus the f32->bf16
cast.  A DRAM round-trip reshuffles (t-part, s) per example into the
DP's (example-part, tau-major) chunk layout.

The state dim is stored MIRRORED (guards at the top) so that 3 of the
4 DP ops have 4-byte-aligned offset-0 operands and hit the DVE 2x_1P
bf16 perf mode (measured: offset-0 non-inplace ops run 2x).

Numerics: bf16 DP state, per-8-step per-example rescale to a 2^43
setpoint (max history written out, logs added back on host), pad states
get all-zero one-hot columns so they decay.  The final splice spans
~e^-180 for tail examples, far outside f32 range, so the two final
state tiles (34KB each) are DMA'd out and the splice runs on host in
f64 (the per-example log + mean were host work already).
"""

import numpy as np

B, T, C, L = 512, 512, 96, 64
BLANK = C - 1
EPS = 1e-7
S = 2 * L + 1          # 129 states
SW = 132               # padded state width (multiple of 4)
NCORES = 8
BN = B // NCORES       # 64 examples per core
TH = T // 2            # split point
RESC = 32              # rescale period
NRESC = (TH - 1) // RESC  # 7 rescales (tau = 32,64,...,224)
SETPOINT_LOG2 = 24     # rescale setpoint 2^24 (headroom for 32 unrescaled steps)
G = 60.646622          # exp(mean_loss/T) boost; keeps alpha ~O(1) per step

_BUILT = None
_LAST_EXEC_NS = None
_LAST_RES = None


def _host_metadata(y_true):
    """ext labels, skip masks, init masks, per-state classes — from y_true.

    Everything is built in natural state order (validated layout), then
    reversed along the free dim at pack time for the mirrored device layout.
    """
    y_true = np.asarray(y_true, dtype=np.int32)
    lbl_len = (y_true != -1).sum(axis=-1).astype(np.int32)
    labels = np.where(y_true != -1, y_true, 0).astype(np.int32)
    ext = np.full((B, S), BLANK, np.int32)
    ext[:, 1::2] = labels
    ext_m2 = np.pad(ext[:, :-2], ((0, 0), (2, 0)), constant_values=BLANK)
    can_skip = ((ext != BLANK) & (ext != ext_m2)).astype(np.float32)

    m2f = np.zeros((B, SW), np.float32)
    m2f[:, :S] = can_skip
    m2b = np.zeros((B, SW), np.float32)
    for u in range(2, S):
        m2b[:, u] = can_skip[:, S - 1 - u + 2]

    mif = np.zeros((B, SW), np.float32)
    mif[:, 0] = 1.0
    mif[:, 1] = 1.0
    mib = np.zeros((B, SW), np.float32)
    mib[np.arange(B), S - 1 - 2 * lbl_len] = 1.0
    mib[np.arange(B), S - 1 - (2 * lbl_len - 1)] = 1.0

    clsf = np.full((B, SW), -1, np.int32)           # -1 -> all-zero column
    clsf[:, :S] = ext
    clsb = np.full((B, SW), -1, np.int32)
    clsb[:, :S] = ext[:, ::-1]
    return m2f, m2b, mif, mib, clsf, clsb


def _build(num_cores=NCORES, t_full=T, bn=BN):
    """Build and schedule the Bass module once."""
    import concourse.bacc as bacc
    import concourse.mybir as mybir
    import concourse.tile as tile
    from contextlib import ExitStack
    from concourse.vector_clock import ScopedClock

    # this walrus build allows a single sem wait per Drain: split the
    # TileContext end-drain's waits across a chain of drains.
    def _patched_drain_and_barrier(self, tick_clock, wait_clock):
        nc = self.nc
        drain_inst = nc.sync.drain()
        wait_clock.add_sem_waits(
            drain_inst.ins, ScopedClock({None: tick_clock.global_clock})
        )
        si = drain_inst.ins.sync_info
        waits = list(si.on_wait) if si and si.on_wait else []
        if len(waits) > 1:
            si.on_wait = waits[:1]
            for w in waits[1:]:
                extra = nc.sync.drain()
                esi = extra.ins.sync_info
                if esi is None:
                    extra.ins.sync_info = mybir.SyncInfo(on_wait=[w], on_update=[])
                else:
                    esi.on_wait = (esi.on_wait or []) + [w]
        nc.all_engine_barrier()
        assert self.sems is not None
        popped = nc._tile_sem_poison_stack.pop()
        assert popped is self._sem_poison
        nc.clear_and_free_semaphores(list(self.sems.allocated().values()))
        nc.all_engine_barrier()

    tile.TileContext._drain_and_barrier = _patched_drain_and_barrier

    f32 = mybir.dt.float32
    bf16 = mybir.dt.bfloat16
    AX = mybir.AxisListType.X
    COPY = mybir.ActivationFunctionType.Copy
    MULT = mybir.AluOpType.mult

    th = t_full // 2
    nblk = t_full // 128
    chk = 32
    nchk = th // chk
    nresc = (th - 1) // RESC

    nc = bacc.Bacc("TRN2", target_bir_lowering=False, debug=False,
                   num_devices=num_cores)
    ypred = nc.dram_tensor("ypred", [bn, t_full, C], f32, kind="ExternalInput")
    m2_in = nc.dram_tensor("m2", [128, SW], bf16, kind="ExternalInput")
    mi_in = nc.dram_tensor("minit", [128, SW], bf16, kind="ExternalInput")
    oh_in = nc.dram_tensor("onehot", [C, bn * 2 * SW], bf16, kind="ExternalInput")
    id_in = nc.dram_tensor("ident", [128, 128], f32, kind="ExternalInput")
    h_out = nc.dram_tensor("hist", [128, max(nresc, 1)], f32, kind="ExternalOutput")
    a_out = nc.dram_tensor("afin", [128, SW + 2], bf16, kind="ExternalOutput")
    g_out = nc.dram_tensor("gfin", [128, SW], bf16, kind="ExternalOutput")

    with tile.TileContext(nc) as tc, ExitStack() as ctx:
        const = ctx.enter_context(tc.tile_pool(name="const", bufs=1))
        dramp = ctx.enter_context(tc.tile_pool(name="edram", bufs=1, space="DRAM"))
        ypf_pool = ctx.enter_context(tc.tile_pool(name="ypf", bufs=3))
        ebf_pool = ctx.enter_context(tc.tile_pool(name="ebf", bufs=2))
        ec_pool = ctx.enter_context(tc.tile_pool(name="ec", bufs=3))
        ytp_pool = ctx.enter_context(tc.tile_pool(name="ytp", bufs=2, space="PSUM"))
        eps_pool = ctx.enter_context(tc.tile_pool(name="eps", bufs=4, space="PSUM"))
        yts_pool = ctx.enter_context(tc.tile_pool(name="yts", bufs=2))

        m2t = const.tile([128, SW], bf16, tag="m2t")
        mit = const.tile([128, SW], bf16, tag="mit")
        oht = const.tile([C, bn * 2 * SW], bf16, tag="oht")
        identt = const.tile([128, 128], f32, tag="identt")
        alpha = const.tile([128, SW + 2], bf16, tag="alpha")
        ut = const.tile([128, SW], bf16, tag="ut")
        vt = const.tile([128, SW], bf16, tag="vt")
        wt = const.tile([128, SW], bf16, tag="wt")
        histt = const.tile([128, max(nresc, 1)], f32, tag="histt")
        sclt = const.tile([128, 1], f32, tag="sclt")

        nc.sync.dma_start(out=m2t[:, :], in_=m2_in.ap())
        nc.sync.dma_start(out=mit[:, :], in_=mi_in.ap())
        gw = 16 * 2 * SW
        for g in range(bn // 16):
            nc.sync.dma_start(out=oht[:, g * gw:(g + 1) * gw],
                              in_=oh_in.ap()[:, g * gw:(g + 1) * gw])
        nc.sync.dma_start(out=identt[:, :], in_=id_in.ap())
        nc.vector.memset(histt[:, :], 0.0)
        nc.vector.memset(alpha[:, :], 0.0)

        # ---- phase A: emissions per 128-t block via PE one-hot matmul ----
        blk_order = []
        for i in range(nblk // 2):
            blk_order += [i, nblk - 1 - i]
        edram = {}
        for k in blk_order:
            ebf_k = ebf_pool.tile([128, bn * SW], bf16)
            dirn = 0 if k < nblk // 2 else 1
            for grp in range(bn // 16):
                ypf = ypf_pool.tile([128, 16 * C], f32)
                yp3 = ypf[:, :].rearrange("p (e c) -> p e c", c=C)
                src = ypred.ap()[grp * 16:(grp + 1) * 16,
                                 k * 128:(k + 1) * 128, :]
                nc.sync.dma_start(out=yp3[:, :, :],
                                  in_=src.rearrange("e t c -> t e c"))
                for q in range(4):          # 4 examples per PSUM group
                    ytp = ytp_pool.tile([C, 512], f32)      # one bank
                    for e4 in range(4):
                        e = q * 4 + e4
                        nc.tensor.transpose(
                            ytp[:, e4 * 128:(e4 + 1) * 128],
                            ypf[:, e * C:(e + 1) * C],
                            identt[:, :])
                    yts = yts_pool.tile([C, 512], bf16)
                    nc.scalar.activation(yts[:, :], ytp[:, :], COPY,
                                         bias=0.0, scale=1.0)
                    for e4 in range(4):
                        e = q * 4 + e4
                        ex = grp * 16 + e
                        ohoff = (ex * 2 + dirn) * SW
                        epsum = eps_pool.tile([128, SW], f32)
                        nc.tensor.matmul(
                            epsum[:, :],
                            yts[:, e4 * 128:(e4 + 1) * 128],
                            oht[:, ohoff:ohoff + SW],
                            start=True, stop=True)
                        nc.scalar.activation(
                            ebf_k[:, ex * SW:(ex + 1) * SW], epsum[:, :],
                            COPY, bias=float(G * EPS), scale=float(G))
            ed = dramp.tile([128, bn * SW], bf16, tag=f"ed{k}")
            edram[k] = ed
            # (t-part, ex*SW contiguous) -> contiguous 16.9KB per partition row
            nc.gpsimd.dma_start(out=ed[:, :], in_=ebf_k[:, :])

        # ---- phase B: reshuffle DRAM -> (example|dir partition, tau) chunks ----
        ec_tiles = []
        for j in range(nchk):
            ec = ec_pool.tile([128, chk * SW], bf16)
            ec3 = ec[:, :].rearrange("p (t s) -> p t s", s=SW)
            kf = j // 4
            tl0 = (j % 4) * chk
            kb = nblk - 1 - kf
            tb0 = 127 - (j % 4) * chk
            fsrc = edram[kf][:, :].rearrange("t (e s) -> t e s", s=SW)
            bsrc = edram[kb][:, :].rearrange("t (e s) -> t e s", s=SW)
            bslice = slice(tb0, None, -1) if tb0 - chk < 0 else slice(tb0, tb0 - chk, -1)
            nc.sync.dma_start(
                out=ec3[0:64, :, :],
                in_=fsrc[tl0:tl0 + chk, :, :].rearrange("t e s -> e t s"))
            nc.gpsimd.dma_start(
                out=ec3[64:128, :, :],
                in_=bsrc[bslice, :, :].rearrange("t e s -> e t s"))
            ec_tiles.append(ec)

        # ---- phase C: the DP (mirrored state layout, guards at top) ----
        nc.vector.tensor_mul(alpha[:, 0:SW], ec_tiles[0][:, 0:SW], mit[:, :])
        nr = 0
        for tau in range(1, th):
            ec = ec_tiles[tau // chk]
            off = (tau % chk) * SW
            nc.vector.tensor_add(ut[:, :], alpha[:, 0:SW], alpha[:, 1:1 + SW])
            nc.vector.tensor_mul(vt[:, :], alpha[:, 2:2 + SW], m2t[:, :])
            nc.vector.tensor_add(wt[:, :], ut[:, :], vt[:, :])
            nc.vector.tensor_mul(alpha[:, 0:SW], wt[:, :], ec[:, off:off + SW])
            if tau % RESC == 0 and nr < nresc:
                # max over mirrored real states i in [2, 132) (incl. one pad col)
                nc.vector.reduce_max(histt[:, nr:nr + 1], alpha[:, 2:SW], axis=AX)
                nc.vector.reciprocal_approx_fast(sclt[:, :], histt[:, nr:nr + 1])
                nc.vector.tensor_scalar(alpha[:, 0:SW], alpha[:, 0:SW],
                                        sclt[:, :], float(2.0 ** SETPOINT_LOG2),
                                        MULT, MULT)
                nr += 1

        # ---- final: gamma on bwd rows, dump states (host does f64 splice) ----
        nc.vector.tensor_add(ut[:, :], alpha[:, 0:SW], alpha[:, 1:1 + SW])
        nc.vector.tensor_mul(vt[:, :], alpha[:, 2:2 + SW], m2t[:, :])
        nc.vector.tensor_add(wt[:, :], ut[:, :], vt[:, :])
        nc.sync.dma_start(out=a_out.ap(), in_=alpha[:, :])
        nc.sync.dma_start(out=g_out.ap(), in_=wt[:, :])
        nc.sync.dma_start(out=h_out.ap(), in_=histt[:, :])

    nc.compile()
    return nc


def kernel(y_true, y_pred):
    global _BUILT, _LAST_EXEC_NS, _LAST_RES
    from concourse.bass_utils import run_bass_kernel_spmd

    y_true = np.asarray(y_true)
    y_pred = np.ascontiguousarray(np.asarray(y_pred, dtype=np.float32))

    m2f, m2b, mif, mib, clsf, clsb = _host_metadata(y_true)

    if _BUILT is None:
        _BUILT = _build()
    nc = _BUILT

    import ml_dtypes
    bf = ml_dtypes.bfloat16
    ident = np.eye(128, dtype=np.float32)
    in_maps = []
    for c in range(NCORES):
        sl = slice(c * BN, (c + 1) * BN)
        # mirrored layout: reverse the free (state) dim
        m2 = np.concatenate([m2f[sl], m2b[sl]], axis=0)[:, ::-1].astype(bf)
        mi = np.concatenate([mif[sl], mib[sl]], axis=0)[:, ::-1].astype(bf)
        oh = np.zeros((C, BN * 2 * SW), bf)
        for e in range(BN):
            b = c * BN + e
            for dirn, cls in ((0, clsf[b]), (1, clsb[b])):
                colbase = (e * 2 + dirn) * SW
                rcls = cls[::-1]                    # mirrored
                for i in range(SW):
                    if rcls[i] >= 0:
                        oh[rcls[i], colbase + i] = bf(1.0)
        in_maps.append({
            "ypred": y_pred[sl],
            "m2": np.ascontiguousarray(m2),
            "minit": np.ascontiguousarray(mi),
            "onehot": oh,
            "ident": ident,
        })

    import os
    trace = os.environ.get("CTC_TRACE", "") == "1"
    res = run_bass_kernel_spmd(nc, in_maps, list(range(NCORES)), trace=trace)
    _LAST_EXEC_NS = res.exec_time_ns
    _LAST_RES = res

    losses = np.zeros(B, np.float64)
    lng = np.log(np.float64(G))
    setlog = NRESC * SETPOINT_LOG2 * np.log(2.0)
    for c in range(NCORES):
        afin = res.results[c]["afin"].astype(np.float64)   # (128, SW+2) mirrored
        gfin = res.results[c]["gfin"].astype(np.float64)   # (128, SW) mirrored
        hist = res.results[c]["hist"].astype(np.float64)
        acc = np.log(np.maximum(hist[:, :NRESC], 1e-300)).sum(axis=1)
        afs = afin[:, 0:SW][:, ::-1]             # un-mirror -> natural state order
        gfs = gfin[:, :][:, ::-1]
        af = afs[0:64, 0:S]                      # alpha_{T/2-1}[s]
        gm = gfs[64:128, 0:S][:, ::-1]           # W_{T/2-1}[s], u -> s
        P = (af * gm).sum(axis=1)
        lnP = np.log(np.maximum(P, 1e-300))
        losses[c * BN:(c + 1) * BN] = -(
            lnP + acc[:64] + acc[64:128] - 2 * setlog - T * lng)
    return np.float32(losses.mean())



# revision 25
# speedup vs baseline: 1.1670x; 1.1615x over previous
"""CTC loss kernel for Trainium2 (8 NeuronCores, data-parallel over batch).

Strategy
--------
Per core: 64 examples. The CTC forward DP runs in probability space
(4 tensor ops per time step on DVE) with states in the free dim and
(example, direction) packed into the 128 partitions: rows 0-63 run the
forward DP for t=0..255, rows 64-127 run the suffix (backward) DP in
state-reversed coordinates for t=511..256.  The two halves are spliced
at T/2:  P = sum_s alpha_255[s] * W_255[s].

Emissions E[b,t,s] = g*(y_pred[b,t,ext_b[s]] + eps) are produced on the
TensorEngine: per (example, 128-t block), PE-transpose y_pred to
(class, t), then a one-hot matmul (ypT.T @ onehot_b) gathers all 132
state emissions for 128 t steps in one instruction.  The scalar engine
copies PSUM->SBUF fusing the g scale and g*eps bias plus the f32->bf16
cast.  A DRAM round-trip reshuffles (t-part, s) per example into the
DP's (example-part, tau-major) chunk layout.

The state dim is stored MIRRORED (guards at the top) so that 3 of the
4 DP ops have 4-byte-aligned offset-0 operands and hit the DVE 2x_1P
bf16 perf mode (measured: offset-0 non-inplace ops run 2x).

Numerics: bf16 DP state, per-8-step per-example rescale to a 2^43
setpoint (max history written out, logs added back on host), pad states
get all-zero one-hot columns so they decay.  The final splice spans
~e^-180 for tail examples, far outside f32 range, so the two final
state tiles (34KB each) are DMA'd out and the splice runs on host in
f64 (the per-example log + mean were host work already).
"""

import numpy as np

B, T, C, L = 512, 512, 96, 64
BLANK = C - 1
EPS = 1e-7
S = 2 * L + 1          # 129 states
SW = 132               # padded state width (multiple of 4)
NCORES = 8
BN = B // NCORES       # 64 examples per core
TH = T // 2            # split point
RESC = 32              # rescale period
NRESC = (TH - 1) // RESC  # 7 rescales (tau = 32,64,...,224)
SETPOINT_LOG2 = 24     # rescale setpoint 2^24 (headroom for 32 unrescaled steps)
G = 60.646622          # exp(mean_loss/T) boost; keeps alpha ~O(1) per step

_BUILT = None
_LAST_EXEC_NS = None
_LAST_RES = None


def _host_metadata(y_true):
    """ext labels, skip masks, init masks, per-state classes — from y_true.

    Everything is built in natural state order (validated layout), then
    reversed along the free dim at pack time for the mirrored device layout.
    """
    y_true = np.asarray(y_true, dtype=np.int32)
    lbl_len = (y_true != -1).sum(axis=-1).astype(np.int32)
    labels = np.where(y_true != -1, y_true, 0).astype(np.int32)
    ext = np.full((B, S), BLANK, np.int32)
    ext[:, 1::2] = labels
    ext_m2 = np.pad(ext[:, :-2], ((0, 0), (2, 0)), constant_values=BLANK)
    can_skip = ((ext != BLANK) & (ext != ext_m2)).astype(np.float32)

    m2f = np.zeros((B, SW), np.float32)
    m2f[:, :S] = can_skip
    m2b = np.zeros((B, SW), np.float32)
    for u in range(2, S):
        m2b[:, u] = can_skip[:, S - 1 - u + 2]

    mif = np.zeros((B, SW), np.float32)
    mif[:, 0] = 1.0
    mif[:, 1] = 1.0
    mib = np.zeros((B, SW), np.float32)
    mib[np.arange(B), S - 1 - 2 * lbl_len] = 1.0
    mib[np.arange(B), S - 1 - (2 * lbl_len - 1)] = 1.0

    clsf = np.full((B, SW), -1, np.int32)           # -1 -> all-zero column
    clsf[:, :S] = ext
    clsb = np.full((B, SW), -1, np.int32)
    clsb[:, :S] = ext[:, ::-1]
    return m2f, m2b, mif, mib, clsf, clsb


def _build(num_cores=NCORES, t_full=T, bn=BN):
    """Build and schedule the Bass module once."""
    import concourse.bacc as bacc
    import concourse.mybir as mybir
    import concourse.tile as tile
    from contextlib import ExitStack
    from concourse.vector_clock import ScopedClock

    # this walrus build allows a single sem wait per Drain: split the
    # TileContext end-drain's waits across a chain of drains.
    def _patched_drain_and_barrier(self, tick_clock, wait_clock):
        nc = self.nc
        drain_inst = nc.sync.drain()
        wait_clock.add_sem_waits(
            drain_inst.ins, ScopedClock({None: tick_clock.global_clock})
        )
        si = drain_inst.ins.sync_info
        waits = list(si.on_wait) if si and si.on_wait else []
        if len(waits) > 1:
            si.on_wait = waits[:1]
            for w in waits[1:]:
                extra = nc.sync.drain()
                esi = extra.ins.sync_info
                if esi is None:
                    extra.ins.sync_info = mybir.SyncInfo(on_wait=[w], on_update=[])
                else:
                    esi.on_wait = (esi.on_wait or []) + [w]
        nc.all_engine_barrier()
        assert self.sems is not None
        popped = nc._tile_sem_poison_stack.pop()
        assert popped is self._sem_poison
        nc.clear_and_free_semaphores(list(self.sems.allocated().values()))
        nc.all_engine_barrier()

    tile.TileContext._drain_and_barrier = _patched_drain_and_barrier

    f32 = mybir.dt.float32
    bf16 = mybir.dt.bfloat16
    AX = mybir.AxisListType.X
    COPY = mybir.ActivationFunctionType.Copy
    MULT = mybir.AluOpType.mult

    th = t_full // 2
    nblk = t_full // 128
    chk = 32
    nchk = th // chk
    nresc = (th - 1) // RESC

    nc = bacc.Bacc("TRN2", target_bir_lowering=False, debug=False,
                   num_devices=num_cores)
    ypred = nc.dram_tensor("ypred", [bn, t_full, C], f32, kind="ExternalInput")
    m2_in = nc.dram_tensor("m2", [128, SW], bf16, kind="ExternalInput")
    mi_in = nc.dram_tensor("minit", [128, SW], bf16, kind="ExternalInput")
    oh_in = nc.dram_tensor("onehot", [C, bn * 2 * SW], bf16, kind="ExternalInput")
    id_in = nc.dram_tensor("ident", [128, 128], bf16, kind="ExternalInput")
    h_out = nc.dram_tensor("hist", [128, max(nresc, 1)], f32, kind="ExternalOutput")
    a_out = nc.dram_tensor("afin", [128, SW + 2], bf16, kind="ExternalOutput")
    g_out = nc.dram_tensor("gfin", [128, SW], bf16, kind="ExternalOutput")

    with tile.TileContext(nc) as tc, ExitStack() as ctx:
        const = ctx.enter_context(tc.tile_pool(name="const", bufs=1))
        dramp = ctx.enter_context(tc.tile_pool(name="edram", bufs=1, space="DRAM"))
        ypf_pool = ctx.enter_context(tc.tile_pool(name="ypf", bufs=3))
        ypb_pool = ctx.enter_context(tc.tile_pool(name="ypb", bufs=3))
        ebf_pool = ctx.enter_context(tc.tile_pool(name="ebf", bufs=2))
        ec_pool = ctx.enter_context(tc.tile_pool(name="ec", bufs=3))
        ytp_pool = ctx.enter_context(tc.tile_pool(name="ytp", bufs=2, space="PSUM"))
        eps_pool = ctx.enter_context(tc.tile_pool(name="eps", bufs=4, space="PSUM"))
        yts_pool = ctx.enter_context(tc.tile_pool(name="yts", bufs=2))

        m2t = const.tile([128, SW], bf16, tag="m2t")
        mit = const.tile([128, SW], bf16, tag="mit")
        oht = const.tile([C, bn * 2 * SW], bf16, tag="oht")
        identt = const.tile([128, 128], bf16, tag="identt")
        alpha = const.tile([128, SW + 2], bf16, tag="alpha")
        ut = const.tile([128, SW], bf16, tag="ut")
        vt = const.tile([128, SW], bf16, tag="vt")
        wt = const.tile([128, SW], bf16, tag="wt")
        histt = const.tile([128, max(nresc, 1)], f32, tag="histt")
        sclt = const.tile([128, 1], f32, tag="sclt")

        nc.sync.dma_start(out=m2t[:, :], in_=m2_in.ap())
        nc.sync.dma_start(out=mit[:, :], in_=mi_in.ap())
        gw = 16 * 2 * SW
        for g in range(bn // 16):
            nc.sync.dma_start(out=oht[:, g * gw:(g + 1) * gw],
                              in_=oh_in.ap()[:, g * gw:(g + 1) * gw])
        nc.sync.dma_start(out=identt[:, :], in_=id_in.ap())
        nc.vector.memset(histt[:, :], 0.0)
        nc.vector.memset(alpha[:, :], 0.0)

        # ---- phase A: emissions per 128-t block via PE one-hot matmul ----
        blk_order = []
        for i in range(nblk // 2):
            blk_order += [i, nblk - 1 - i]
        edram = {}
        for kidx, k in enumerate(blk_order):
            ebf_k = ebf_pool.tile([128, bn * SW], bf16)
            dirn = 0 if k < nblk // 2 else 1
            early = kidx < 2          # blocks {0,3}: cast on idle DVE
            for grp in range(bn // 16):
                ypf = ypf_pool.tile([128, 16 * C], f32)
                yp3 = ypf[:, :].rearrange("p (e c) -> p e c", c=C)
                src = ypred.ap()[grp * 16:(grp + 1) * 16,
                                 k * 128:(k + 1) * 128, :]
                nc.sync.dma_start(out=yp3[:, :, :],
                                  in_=src.rearrange("e t c -> t e c"))
                ypb = ypb_pool.tile([128, 16 * C], bf16)
                if early:
                    nc.vector.tensor_copy(ypb[:, :], ypf[:, :])
                else:
                    nc.gpsimd.tensor_copy(ypb[:, :], ypf[:, :])
                for q in range(4):          # 4 examples per PSUM group
                    ytp = ytp_pool.tile([C, 512], bf16)     # half a bank
                    for e4 in range(4):
                        e = q * 4 + e4
                        nc.tensor.transpose(
                            ytp[:, e4 * 128:(e4 + 1) * 128],
                            ypb[:, e * C:(e + 1) * C],
                            identt[:, :])
                    yts = yts_pool.tile([C, 512], bf16)
                    nc.scalar.activation(yts[:, :], ytp[:, :], COPY,
                                         bias=0.0, scale=1.0)
                    for e4 in range(4):
                        e = q * 4 + e4
                        ex = grp * 16 + e
                        ohoff = (ex * 2 + dirn) * SW
                        epsum = eps_pool.tile([128, SW], f32)
                        nc.tensor.matmul(
                            epsum[:, :],
                            yts[:, e4 * 128:(e4 + 1) * 128],
                            oht[:, ohoff:ohoff + SW],
                            start=True, stop=True)
                        nc.scalar.activation(
                            ebf_k[:, ex * SW:(ex + 1) * SW], epsum[:, :],
                            COPY, bias=float(G * EPS), scale=float(G))
            ed = dramp.tile([128, bn * SW], bf16, tag=f"ed{k}")
            edram[k] = ed
            # (t-part, ex*SW contiguous) -> contiguous 16.9KB per partition row
            nc.gpsimd.dma_start(out=ed[:, :], in_=ebf_k[:, :])

        # ---- phase B: reshuffle DRAM -> (example|dir partition, tau) chunks ----
        ec_tiles = []
        for j in range(nchk):
            ec = ec_pool.tile([128, chk * SW], bf16)
            ec3 = ec[:, :].rearrange("p (t s) -> p t s", s=SW)
            kf = j // 4
            tl0 = (j % 4) * chk
            kb = nblk - 1 - kf
            tb0 = 127 - (j % 4) * chk
            fsrc = edram[kf][:, :].rearrange("t (e s) -> t e s", s=SW)
            bsrc = edram[kb][:, :].rearrange("t (e s) -> t e s", s=SW)
            bslice = slice(tb0, None, -1) if tb0 - chk < 0 else slice(tb0, tb0 - chk, -1)
            nc.sync.dma_start(
                out=ec3[0:64, :, :],
                in_=fsrc[tl0:tl0 + chk, :, :].rearrange("t e s -> e t s"))
            nc.gpsimd.dma_start(
                out=ec3[64:128, :, :],
                in_=bsrc[bslice, :, :].rearrange("t e s -> e t s"))
            ec_tiles.append(ec)

        # ---- phase C: the DP (mirrored state layout, guards at top) ----
        nc.vector.tensor_mul(alpha[:, 0:SW], ec_tiles[0][:, 0:SW], mit[:, :])
        nr = 0
        for tau in range(1, th):
            ec = ec_tiles[tau // chk]
            off = (tau % chk) * SW
            nc.vector.tensor_add(ut[:, :], alpha[:, 0:SW], alpha[:, 1:1 + SW])
            nc.vector.tensor_mul(vt[:, :], alpha[:, 2:2 + SW], m2t[:, :])
            nc.vector.tensor_add(wt[:, :], ut[:, :], vt[:, :])
            nc.vector.tensor_mul(alpha[:, 0:SW], wt[:, :], ec[:, off:off + SW])
            if tau % RESC == 0 and nr < nresc:
                # max over mirrored real states i in [2, 132) (incl. one pad col)
                nc.vector.reduce_max(histt[:, nr:nr + 1], alpha[:, 2:SW], axis=AX)
                nc.vector.reciprocal_approx_fast(sclt[:, :], histt[:, nr:nr + 1])
                nc.vector.tensor_scalar(alpha[:, 0:SW], alpha[:, 0:SW],
                                        sclt[:, :], float(2.0 ** SETPOINT_LOG2),
                                        MULT, MULT)
                nr += 1

        # ---- final: gamma on bwd rows, dump states (host does f64 splice) ----
        nc.vector.tensor_add(ut[:, :], alpha[:, 0:SW], alpha[:, 1:1 + SW])
        nc.vector.tensor_mul(vt[:, :], alpha[:, 2:2 + SW], m2t[:, :])
        nc.vector.tensor_add(wt[:, :], ut[:, :], vt[:, :])
        nc.sync.dma_start(out=a_out.ap(), in_=alpha[:, :])
        nc.sync.dma_start(out=g_out.ap(), in_=wt[:, :])
        nc.sync.dma_start(out=h_out.ap(), in_=histt[:, :])

    nc.compile()
    return nc


def kernel(y_true, y_pred):
    global _BUILT, _LAST_EXEC_NS, _LAST_RES
    from concourse.bass_utils import run_bass_kernel_spmd

    y_true = np.asarray(y_true)
    y_pred = np.ascontiguousarray(np.asarray(y_pred, dtype=np.float32))

    m2f, m2b, mif, mib, clsf, clsb = _host_metadata(y_true)

    if _BUILT is None:
        _BUILT = _build()
    nc = _BUILT

    import ml_dtypes
    bf = ml_dtypes.bfloat16
    ident = np.eye(128, dtype=np.float32)
    in_maps = []
    for c in range(NCORES):
        sl = slice(c * BN, (c + 1) * BN)
        # mirrored layout: reverse the free (state) dim
        m2 = np.concatenate([m2f[sl], m2b[sl]], axis=0)[:, ::-1].astype(bf)
        mi = np.concatenate([mif[sl], mib[sl]], axis=0)[:, ::-1].astype(bf)
        oh = np.zeros((C, BN * 2 * SW), bf)
        for e in range(BN):
            b = c * BN + e
            for dirn, cls in ((0, clsf[b]), (1, clsb[b])):
                colbase = (e * 2 + dirn) * SW
                rcls = cls[::-1]                    # mirrored
                for i in range(SW):
                    if rcls[i] >= 0:
                        oh[rcls[i], colbase + i] = bf(1.0)
        in_maps.append({
            "ypred": y_pred[sl],
            "m2": np.ascontiguousarray(m2),
            "minit": np.ascontiguousarray(mi),
            "onehot": oh,
            "ident": ident.astype(bf),
        })

    import os
    trace = os.environ.get("CTC_TRACE", "") == "1"
    res = run_bass_kernel_spmd(nc, in_maps, list(range(NCORES)), trace=trace)
    _LAST_EXEC_NS = res.exec_time_ns
    _LAST_RES = res

    losses = np.zeros(B, np.float64)
    lng = np.log(np.float64(G))
    setlog = NRESC * SETPOINT_LOG2 * np.log(2.0)
    for c in range(NCORES):
        afin = res.results[c]["afin"].astype(np.float64)   # (128, SW+2) mirrored
        gfin = res.results[c]["gfin"].astype(np.float64)   # (128, SW) mirrored
        hist = res.results[c]["hist"].astype(np.float64)
        acc = np.log(np.maximum(hist[:, :NRESC], 1e-300)).sum(axis=1)
        afs = afin[:, 0:SW][:, ::-1]             # un-mirror -> natural state order
        gfs = gfin[:, :][:, ::-1]
        af = afs[0:64, 0:S]                      # alpha_{T/2-1}[s]
        gm = gfs[64:128, 0:S][:, ::-1]           # W_{T/2-1}[s], u -> s
        P = (af * gm).sum(axis=1)
        lnP = np.log(np.maximum(P, 1e-300))
        losses[c * BN:(c + 1) * BN] = -(
            lnP + acc[:64] + acc[64:128] - 2 * setlog - T * lng)
    return np.float32(losses.mean())



# revision 26
# speedup vs baseline: 1.1825x; 1.0134x over previous
"""CTC loss kernel for Trainium2 (8 NeuronCores, data-parallel over batch).

Strategy
--------
Per core: 64 examples. The CTC forward DP runs in probability space
(4 tensor ops per time step on DVE) with states in the free dim and
(example, direction) packed into the 128 partitions: rows 0-63 run the
forward DP for t=0..255, rows 64-127 run the suffix (backward) DP in
state-reversed coordinates for t=511..256.  The two halves are spliced
at T/2:  P = sum_s alpha_255[s] * W_255[s].

Emissions E[b,t,s] = g*(y_pred[b,t,ext_b[s]] + eps) are produced on the
TensorEngine: per (example, 128-t block), PE-transpose y_pred to
(class, t), then a one-hot matmul (ypT.T @ onehot_b) gathers all 132
state emissions for 128 t steps in one instruction.  The scalar engine
copies PSUM->SBUF fusing the g scale and g*eps bias plus the f32->bf16
cast.  A DRAM round-trip reshuffles (t-part, s) per example into the
DP's (example-part, tau-major) chunk layout.

The state dim is stored MIRRORED (guards at the top) so that 3 of the
4 DP ops have 4-byte-aligned offset-0 operands and hit the DVE 2x_1P
bf16 perf mode (measured: offset-0 non-inplace ops run 2x).

Numerics: bf16 DP state, per-8-step per-example rescale to a 2^43
setpoint (max history written out, logs added back on host), pad states
get all-zero one-hot columns so they decay.  The final splice spans
~e^-180 for tail examples, far outside f32 range, so the two final
state tiles (34KB each) are DMA'd out and the splice runs on host in
f64 (the per-example log + mean were host work already).
"""

import numpy as np

B, T, C, L = 512, 512, 96, 64
BLANK = C - 1
EPS = 1e-7
S = 2 * L + 1          # 129 states
SW = 132               # padded state width (multiple of 4)
NCORES = 8
BN = B // NCORES       # 64 examples per core
TH = T // 2            # split point
RESC = 32              # rescale period
NRESC = (TH - 1) // RESC  # 7 rescales (tau = 32,64,...,224)
SETPOINT_LOG2 = 24     # rescale setpoint 2^24 (headroom for 32 unrescaled steps)
G = 60.646622          # exp(mean_loss/T) boost; keeps alpha ~O(1) per step

_BUILT = None
_LAST_EXEC_NS = None
_LAST_RES = None


def _host_metadata(y_true):
    """ext labels, skip masks, init masks, per-state classes — from y_true.

    Everything is built in natural state order (validated layout), then
    reversed along the free dim at pack time for the mirrored device layout.
    """
    y_true = np.asarray(y_true, dtype=np.int32)
    lbl_len = (y_true != -1).sum(axis=-1).astype(np.int32)
    labels = np.where(y_true != -1, y_true, 0).astype(np.int32)
    ext = np.full((B, S), BLANK, np.int32)
    ext[:, 1::2] = labels
    ext_m2 = np.pad(ext[:, :-2], ((0, 0), (2, 0)), constant_values=BLANK)
    can_skip = ((ext != BLANK) & (ext != ext_m2)).astype(np.float32)

    m2f = np.zeros((B, SW), np.float32)
    m2f[:, :S] = can_skip
    m2b = np.zeros((B, SW), np.float32)
    for u in range(2, S):
        m2b[:, u] = can_skip[:, S - 1 - u + 2]

    mif = np.zeros((B, SW), np.float32)
    mif[:, 0] = 1.0
    mif[:, 1] = 1.0
    mib = np.zeros((B, SW), np.float32)
    mib[np.arange(B), S - 1 - 2 * lbl_len] = 1.0
    mib[np.arange(B), S - 1 - (2 * lbl_len - 1)] = 1.0

    clsf = np.full((B, SW), -1, np.int32)           # -1 -> all-zero column
    clsf[:, :S] = ext
    clsb = np.full((B, SW), -1, np.int32)
    clsb[:, :S] = ext[:, ::-1]
    return m2f, m2b, mif, mib, clsf, clsb


def _build(num_cores=NCORES, t_full=T, bn=BN):
    """Build and schedule the Bass module once."""
    import concourse.bacc as bacc
    import concourse.mybir as mybir
    import concourse.tile as tile
    from contextlib import ExitStack
    from concourse.vector_clock import ScopedClock

    # this walrus build allows a single sem wait per Drain: split the
    # TileContext end-drain's waits across a chain of drains.
    def _patched_drain_and_barrier(self, tick_clock, wait_clock):
        nc = self.nc
        drain_inst = nc.sync.drain()
        wait_clock.add_sem_waits(
            drain_inst.ins, ScopedClock({None: tick_clock.global_clock})
        )
        si = drain_inst.ins.sync_info
        waits = list(si.on_wait) if si and si.on_wait else []
        if len(waits) > 1:
            si.on_wait = waits[:1]
            for w in waits[1:]:
                extra = nc.sync.drain()
                esi = extra.ins.sync_info
                if esi is None:
                    extra.ins.sync_info = mybir.SyncInfo(on_wait=[w], on_update=[])
                else:
                    esi.on_wait = (esi.on_wait or []) + [w]
        nc.all_engine_barrier()
        assert self.sems is not None
        popped = nc._tile_sem_poison_stack.pop()
        assert popped is self._sem_poison
        nc.clear_and_free_semaphores(list(self.sems.allocated().values()))
        nc.all_engine_barrier()

    tile.TileContext._drain_and_barrier = _patched_drain_and_barrier

    f32 = mybir.dt.float32
    bf16 = mybir.dt.bfloat16
    AX = mybir.AxisListType.X
    COPY = mybir.ActivationFunctionType.Copy
    MULT = mybir.AluOpType.mult

    th = t_full // 2
    nblk = t_full // 128
    chk = 32
    nchk = th // chk
    nresc = (th - 1) // RESC

    nc = bacc.Bacc("TRN2", target_bir_lowering=False, debug=False,
                   num_devices=num_cores)
    ypred = nc.dram_tensor("ypred", [bn, t_full, C], f32, kind="ExternalInput")
    m2_in = nc.dram_tensor("m2", [128, SW], bf16, kind="ExternalInput")
    mi_in = nc.dram_tensor("minit", [128, SW], bf16, kind="ExternalInput")
    oh_in = nc.dram_tensor("onehot", [C, bn * 2 * SW], bf16, kind="ExternalInput")
    id_in = nc.dram_tensor("ident", [128, 128], f32, kind="ExternalInput")
    h_out = nc.dram_tensor("hist", [128, max(nresc, 1)], f32, kind="ExternalOutput")
    a_out = nc.dram_tensor("afin", [128, SW + 2], bf16, kind="ExternalOutput")
    g_out = nc.dram_tensor("gfin", [128, SW], bf16, kind="ExternalOutput")

    with tile.TileContext(nc) as tc, ExitStack() as ctx:
        const = ctx.enter_context(tc.tile_pool(name="const", bufs=1))
        dramp = ctx.enter_context(tc.tile_pool(name="edram", bufs=1, space="DRAM"))
        ypf_pool = ctx.enter_context(tc.tile_pool(name="ypf", bufs=3))
        ebf_pool = ctx.enter_context(tc.tile_pool(name="ebf", bufs=2))
        ec_pool = ctx.enter_context(tc.tile_pool(name="ec", bufs=3))
        ytp_pool = ctx.enter_context(tc.tile_pool(name="ytp", bufs=2, space="PSUM"))
        eps_pool = ctx.enter_context(tc.tile_pool(name="eps", bufs=4, space="PSUM"))
        yts_pool = ctx.enter_context(tc.tile_pool(name="yts", bufs=2))

        m2t = const.tile([128, SW], bf16, tag="m2t")
        mit = const.tile([128, SW], bf16, tag="mit")
        oht = const.tile([C, bn * 2 * SW], bf16, tag="oht")
        identt = const.tile([128, 128], f32, tag="identt")
        alpha = const.tile([128, SW + 2], bf16, tag="alpha")
        ut = const.tile([128, SW], bf16, tag="ut")
        vt = const.tile([128, SW], bf16, tag="vt")
        wt = const.tile([128, SW], bf16, tag="wt")
        histt = const.tile([128, max(nresc, 1)], f32, tag="histt")
        sclt = const.tile([128, 1], f32, tag="sclt")

        nc.sync.dma_start(out=m2t[:, :], in_=m2_in.ap())
        nc.sync.dma_start(out=mit[:, :], in_=mi_in.ap())
        gw = 16 * 2 * SW
        for g in range(bn // 16):
            nc.sync.dma_start(out=oht[:, g * gw:(g + 1) * gw],
                              in_=oh_in.ap()[:, g * gw:(g + 1) * gw])
        nc.sync.dma_start(out=identt[:, :], in_=id_in.ap())
        nc.vector.memset(histt[:, :], 0.0)
        nc.vector.memset(alpha[:, :], 0.0)

        # ---- phase A: emissions per 128-t block via PE one-hot matmul ----
        blk_order = []
        for i in range(nblk // 2):
            blk_order += [i, nblk - 1 - i]
        edram = {}
        for k in blk_order:
            ebf_k = ebf_pool.tile([128, bn * SW], bf16)
            dirn = 0 if k < nblk // 2 else 1
            for grp in range(bn // 16):
                ypf = ypf_pool.tile([128, 16 * C], f32)
                yp3 = ypf[:, :].rearrange("p (e c) -> p e c", c=C)
                src = ypred.ap()[grp * 16:(grp + 1) * 16,
                                 k * 128:(k + 1) * 128, :]
                nc.sync.dma_start(out=yp3[:, :, :],
                                  in_=src.rearrange("e t c -> t e c"))
                for q in range(4):          # 4 examples per PSUM group
                    ytp = ytp_pool.tile([C, 512], f32)      # one bank
                    for e4 in range(4):
                        e = q * 4 + e4
                        nc.tensor.transpose(
                            ytp[:, e4 * 128:(e4 + 1) * 128],
                            ypf[:, e * C:(e + 1) * C],
                            identt[:, :])
                    yts = yts_pool.tile([C, 512], bf16)
                    nc.scalar.activation(yts[:, :], ytp[:, :], COPY,
                                         bias=0.0, scale=1.0)
                    for e4 in range(4):
                        e = q * 4 + e4
                        ex = grp * 16 + e
                        ohoff = (ex * 2 + dirn) * SW
                        epsum = eps_pool.tile([128, SW], f32)
                        nc.tensor.matmul(
                            epsum[:, :],
                            yts[:, e4 * 128:(e4 + 1) * 128],
                            oht[:, ohoff:ohoff + SW],
                            start=True, stop=True)
                        nc.scalar.activation(
                            ebf_k[:, ex * SW:(ex + 1) * SW], epsum[:, :],
                            COPY, bias=float(G * EPS), scale=float(G))
            ed = dramp.tile([128, bn * SW], bf16, tag=f"ed{k}")
            edram[k] = ed
            # (t-part, ex*SW contiguous) -> contiguous 16.9KB per partition row
            nc.gpsimd.dma_start(out=ed[:, :], in_=ebf_k[:, :])

        # ---- phase B: reshuffle DRAM -> (example|dir partition, tau) chunks ----
        ec_tiles = []
        for j in range(nchk):
            ec = ec_pool.tile([128, chk * SW], bf16)
            ec3 = ec[:, :].rearrange("p (t s) -> p t s", s=SW)
            kf = j // 4
            tl0 = (j % 4) * chk
            kb = nblk - 1 - kf
            tb0 = 127 - (j % 4) * chk
            fsrc = edram[kf][:, :].rearrange("t (e s) -> t e s", s=SW)
            bsrc = edram[kb][:, :].rearrange("t (e s) -> t e s", s=SW)
            bslice = slice(tb0, None, -1) if tb0 - chk < 0 else slice(tb0, tb0 - chk, -1)
            nc.sync.dma_start(
                out=ec3[0:64, :, :],
                in_=fsrc[tl0:tl0 + chk, :, :].rearrange("t e s -> e t s"))
            nc.gpsimd.dma_start(
                out=ec3[64:128, :, :],
                in_=bsrc[bslice, :, :].rearrange("t e s -> e t s"))
            ec_tiles.append(ec)

        # ---- phase C: the DP (mirrored state layout, guards at top) ----
        nc.vector.tensor_mul(alpha[:, 0:SW], ec_tiles[0][:, 0:SW], mit[:, :])
        nr = 0
        for tau in range(1, th):
            ec = ec_tiles[tau // chk]
            off = (tau % chk) * SW
            nc.vector.tensor_add(ut[:, :], alpha[:, 0:SW], alpha[:, 1:1 + SW])
            nc.vector.tensor_mul(vt[:, :], alpha[:, 2:2 + SW], m2t[:, :])
            nc.vector.tensor_add(wt[:, :], ut[:, :], vt[:, :])
            nc.vector.tensor_mul(alpha[:, 0:SW], wt[:, :], ec[:, off:off + SW])
            if tau % RESC == 0 and nr < nresc:
                # max over mirrored real states i in [2, 132) (incl. one pad col)
                nc.vector.reduce_max(histt[:, nr:nr + 1], alpha[:, 2:SW], axis=AX)
                nc.vector.reciprocal_approx_fast(sclt[:, :], histt[:, nr:nr + 1])
                nc.vector.tensor_scalar(alpha[:, 0:SW], alpha[:, 0:SW],
                                        sclt[:, :], float(2.0 ** SETPOINT_LOG2),
                                        MULT, MULT)
                nr += 1

        # ---- final: gamma on bwd rows, dump states (host does f64 splice) ----
        nc.vector.tensor_add(ut[:, :], alpha[:, 0:SW], alpha[:, 1:1 + SW])
        nc.vector.tensor_mul(vt[:, :], alpha[:, 2:2 + SW], m2t[:, :])
        nc.vector.tensor_add(wt[:, :], ut[:, :], vt[:, :])
        nc.sync.dma_start(out=a_out.ap(), in_=alpha[:, :])
        nc.sync.dma_start(out=g_out.ap(), in_=wt[:, :])
        nc.sync.dma_start(out=h_out.ap(), in_=histt[:, :])

    nc.compile()
    return nc


def kernel(y_true, y_pred):
    global _BUILT, _LAST_EXEC_NS, _LAST_RES
    from concourse.bass_utils import run_bass_kernel_spmd

    y_true = np.asarray(y_true)
    y_pred = np.ascontiguousarray(np.asarray(y_pred, dtype=np.float32))

    m2f, m2b, mif, mib, clsf, clsb = _host_metadata(y_true)

    if _BUILT is None:
        _BUILT = _build()
    nc = _BUILT

    import ml_dtypes
    bf = ml_dtypes.bfloat16
    ident = np.eye(128, dtype=np.float32)
    in_maps = []
    for c in range(NCORES):
        sl = slice(c * BN, (c + 1) * BN)
        # mirrored layout: reverse the free (state) dim
        m2 = np.concatenate([m2f[sl], m2b[sl]], axis=0)[:, ::-1].astype(bf)
        mi = np.concatenate([mif[sl], mib[sl]], axis=0)[:, ::-1].astype(bf)
        oh = np.zeros((C, BN * 2 * SW), bf)
        for e in range(BN):
            b = c * BN + e
            for dirn, cls in ((0, clsf[b]), (1, clsb[b])):
                colbase = (e * 2 + dirn) * SW
                rcls = cls[::-1]                    # mirrored
                for i in range(SW):
                    if rcls[i] >= 0:
                        oh[rcls[i], colbase + i] = bf(1.0)
        in_maps.append({
            "ypred": y_pred[sl],
            "m2": np.ascontiguousarray(m2),
            "minit": np.ascontiguousarray(mi),
            "onehot": oh,
            "ident": ident,
        })

    import os
    trace = os.environ.get("CTC_TRACE", "") == "1"
    res = run_bass_kernel_spmd(nc, in_maps, list(range(NCORES)), trace=trace)
    _LAST_EXEC_NS = res.exec_time_ns
    _LAST_RES = res

    losses = np.zeros(B, np.float64)
    lng = np.log(np.float64(G))
    setlog = NRESC * SETPOINT_LOG2 * np.log(2.0)
    for c in range(NCORES):
        afin = res.results[c]["afin"].astype(np.float64)   # (128, SW+2) mirrored
        gfin = res.results[c]["gfin"].astype(np.float64)   # (128, SW) mirrored
        hist = res.results[c]["hist"].astype(np.float64)
        acc = np.log(np.maximum(hist[:, :NRESC], 1e-300)).sum(axis=1)
        afs = afin[:, 0:SW][:, ::-1]             # un-mirror -> natural state order
        gfs = gfin[:, :][:, ::-1]
        af = afs[0:64, 0:S]                      # alpha_{T/2-1}[s]
        gm = gfs[64:128, 0:S][:, ::-1]           # W_{T/2-1}[s], u -> s
        P = (af * gm).sum(axis=1)
        lnP = np.log(np.maximum(P, 1e-300))
        losses[c * BN:(c + 1) * BN] = -(
            lnP + acc[:64] + acc[64:128] - 2 * setlog - T * lng)
    return np.float32(losses.mean())



# revision 28
# speedup vs baseline: 1.2044x; 1.0185x over previous
"""CTC loss kernel for Trainium2 (8 NeuronCores, data-parallel over batch).

Strategy
--------
Per core: 64 examples. The CTC forward DP runs in probability space
(4 tensor ops per time step on DVE) with states in the free dim and
(example, direction) packed into the 128 partitions: rows 0-63 run the
forward DP for t=0..255, rows 64-127 run the suffix (backward) DP in
state-reversed coordinates for t=511..256.  The two halves are spliced
at T/2:  P = sum_s alpha_255[s] * W_255[s].

Emissions E[b,t,s] = g*(y_pred[b,t,ext_b[s]] + eps) are produced on the
TensorEngine: per (example, 128-t block), PE-transpose y_pred to
(class, t), then a one-hot matmul (ypT.T @ onehot_b) gathers all 132
state emissions for 128 t steps in one instruction.  The scalar engine
copies PSUM->SBUF fusing the g scale and g*eps bias plus the f32->bf16
cast.  A DRAM round-trip reshuffles (t-part, s) per example into the
DP's (example-part, tau-major) chunk layout.

The state dim is stored MIRRORED (guards at the top) so that 3 of the
4 DP ops have 4-byte-aligned offset-0 operands and hit the DVE 2x_1P
bf16 perf mode (measured: offset-0 non-inplace ops run 2x).

Numerics: bf16 DP state, per-8-step per-example rescale to a 2^43
setpoint (max history written out, logs added back on host), pad states
get all-zero one-hot columns so they decay.  The final splice spans
~e^-180 for tail examples, far outside f32 range, so the two final
state tiles (34KB each) are DMA'd out and the splice runs on host in
f64 (the per-example log + mean were host work already).
"""

import numpy as np

B, T, C, L = 512, 512, 96, 64
BLANK = C - 1
EPS = 1e-7
S = 2 * L + 1          # 129 states
SW = 132               # padded state width (multiple of 4)
NCORES = 8
BN = B // NCORES       # 64 examples per core
TH = T // 2            # split point
RESC = 32              # rescale period
NRESC = (TH - 1) // RESC  # 7 rescales (tau = 32,64,...,224)
SETPOINT_LOG2 = 24     # rescale setpoint 2^24 (headroom for 32 unrescaled steps)
G = 60.646622          # exp(mean_loss/T) boost; keeps alpha ~O(1) per step

_BUILT = None
_LAST_EXEC_NS = None
_LAST_RES = None


def _host_metadata(y_true):
    """ext labels, skip masks, init masks, per-state classes — from y_true.

    Everything is built in natural state order (validated layout), then
    reversed along the free dim at pack time for the mirrored device layout.
    """
    y_true = np.asarray(y_true, dtype=np.int32)
    lbl_len = (y_true != -1).sum(axis=-1).astype(np.int32)
    labels = np.where(y_true != -1, y_true, 0).astype(np.int32)
    ext = np.full((B, S), BLANK, np.int32)
    ext[:, 1::2] = labels
    ext_m2 = np.pad(ext[:, :-2], ((0, 0), (2, 0)), constant_values=BLANK)
    can_skip = ((ext != BLANK) & (ext != ext_m2)).astype(np.float32)

    m2f = np.zeros((B, SW), np.float32)
    m2f[:, :S] = can_skip
    m2b = np.zeros((B, SW), np.float32)
    for u in range(2, S):
        m2b[:, u] = can_skip[:, S - 1 - u + 2]

    mif = np.zeros((B, SW), np.float32)
    mif[:, 0] = 1.0
    mif[:, 1] = 1.0
    mib = np.zeros((B, SW), np.float32)
    mib[np.arange(B), S - 1 - 2 * lbl_len] = 1.0
    mib[np.arange(B), S - 1 - (2 * lbl_len - 1)] = 1.0

    clsf = np.full((B, SW), -1, np.int32)           # -1 -> all-zero column
    clsf[:, :S] = ext
    clsb = np.full((B, SW), -1, np.int32)
    clsb[:, :S] = ext[:, ::-1]
    return m2f, m2b, mif, mib, clsf, clsb


def _build(num_cores=NCORES, t_full=T, bn=BN):
    """Build and schedule the Bass module once."""
    import concourse.bacc as bacc
    import concourse.mybir as mybir
    import concourse.tile as tile
    from contextlib import ExitStack
    from concourse.vector_clock import ScopedClock

    # this walrus build allows a single sem wait per Drain: split the
    # TileContext end-drain's waits across a chain of drains.
    def _patched_drain_and_barrier(self, tick_clock, wait_clock):
        nc = self.nc
        drain_inst = nc.sync.drain()
        wait_clock.add_sem_waits(
            drain_inst.ins, ScopedClock({None: tick_clock.global_clock})
        )
        si = drain_inst.ins.sync_info
        waits = list(si.on_wait) if si and si.on_wait else []
        if len(waits) > 1:
            si.on_wait = waits[:1]
            for w in waits[1:]:
                extra = nc.sync.drain()
                esi = extra.ins.sync_info
                if esi is None:
                    extra.ins.sync_info = mybir.SyncInfo(on_wait=[w], on_update=[])
                else:
                    esi.on_wait = (esi.on_wait or []) + [w]
        nc.all_engine_barrier()
        assert self.sems is not None
        popped = nc._tile_sem_poison_stack.pop()
        assert popped is self._sem_poison
        nc.clear_and_free_semaphores(list(self.sems.allocated().values()))
        nc.all_engine_barrier()

    tile.TileContext._drain_and_barrier = _patched_drain_and_barrier

    f32 = mybir.dt.float32
    bf16 = mybir.dt.bfloat16
    AX = mybir.AxisListType.X
    COPY = mybir.ActivationFunctionType.Copy
    MULT = mybir.AluOpType.mult

    th = t_full // 2
    nblk = t_full // 128
    chk = 32
    nchk = th // chk
    nresc = (th - 1) // RESC

    nc = bacc.Bacc("TRN2", target_bir_lowering=False, debug=False,
                   num_devices=num_cores)
    ypred = nc.dram_tensor("ypred", [bn, t_full, C], f32, kind="ExternalInput")
    m2_in = nc.dram_tensor("m2", [128, SW], bf16, kind="ExternalInput")
    mi_in = nc.dram_tensor("minit", [128, SW], bf16, kind="ExternalInput")
    oh_in = nc.dram_tensor("onehot", [C, bn * 2 * SW], bf16, kind="ExternalInput")
    id_in = nc.dram_tensor("ident", [128, 128], f32, kind="ExternalInput")
    h_out = nc.dram_tensor("hist", [128, max(nresc, 1)], f32, kind="ExternalOutput")
    a_out = nc.dram_tensor("afin", [128, SW + 2], bf16, kind="ExternalOutput")
    g_out = nc.dram_tensor("gfin", [128, SW], bf16, kind="ExternalOutput")

    with tile.TileContext(nc) as tc, ExitStack() as ctx:
        const = ctx.enter_context(tc.tile_pool(name="const", bufs=1))
        dramp = ctx.enter_context(tc.tile_pool(name="edram", bufs=1, space="DRAM"))
        ypf_pool = ctx.enter_context(tc.tile_pool(name="ypf", bufs=3))
        ebf_pool = ctx.enter_context(tc.tile_pool(name="ebf", bufs=2))
        ec_pool = ctx.enter_context(tc.tile_pool(name="ec", bufs=4))
        ytp_pool = ctx.enter_context(tc.tile_pool(name="ytp", bufs=2, space="PSUM"))
        eps_pool = ctx.enter_context(tc.tile_pool(name="eps", bufs=4, space="PSUM"))
        yts_pool = ctx.enter_context(tc.tile_pool(name="yts", bufs=2))

        m2t = const.tile([128, SW], bf16, tag="m2t")
        mit = const.tile([128, SW], bf16, tag="mit")
        oht = const.tile([C, bn * 2 * SW], bf16, tag="oht")
        identt = const.tile([128, 128], f32, tag="identt")
        alpha = const.tile([128, SW + 2], bf16, tag="alpha")
        ut = const.tile([128, SW], bf16, tag="ut")
        vt = const.tile([128, SW], bf16, tag="vt")
        wt = const.tile([128, SW], bf16, tag="wt")
        histt = const.tile([128, max(nresc, 1)], f32, tag="histt")
        sclt = const.tile([128, 1], f32, tag="sclt")

        blk_order = []
        for i in range(nblk // 2):
            blk_order += [i, nblk - 1 - i]

        def _load_ypf(k, grp):
            ypf = ypf_pool.tile([128, 16 * C], f32)
            yp3 = ypf[:, :].rearrange("p (e c) -> p e c", c=C)
            src = ypred.ap()[grp * 16:(grp + 1) * 16,
                             k * 128:(k + 1) * 128, :]
            nc.sync.dma_start(out=yp3[:, :, :],
                              in_=src.rearrange("e t c -> t e c"))
            return ypf

        # first y_pred loads + identity BEFORE the bulky const DMAs so the
        # PE pipeline starts immediately
        prefetched = {(blk_order[0], 0): _load_ypf(blk_order[0], 0)}
        nc.sync.dma_start(out=identt[:, :], in_=id_in.ap())
        prefetched[(blk_order[0], 1)] = _load_ypf(blk_order[0], 1)
        nc.sync.dma_start(out=m2t[:, :], in_=m2_in.ap())
        nc.sync.dma_start(out=mit[:, :], in_=mi_in.ap())
        gw = 16 * 2 * SW
        for g in range(bn // 16):
            nc.sync.dma_start(out=oht[:, g * gw:(g + 1) * gw],
                              in_=oh_in.ap()[:, g * gw:(g + 1) * gw])
        nc.vector.memset(histt[:, :], 0.0)
        nc.vector.memset(alpha[:, :], 0.0)

        # ---- phase A: emissions per 128-t block via PE one-hot matmul ----
        edram = {}
        for k in blk_order:
            ebf_k = ebf_pool.tile([128, bn * SW], bf16)
            dirn = 0 if k < nblk // 2 else 1
            for grp in range(bn // 16):
                ypf = prefetched.pop((k, grp), None)
                if ypf is None:
                    ypf = _load_ypf(k, grp)
                for q in range(4):          # 4 examples per PSUM group
                    ytp = ytp_pool.tile([C, 512], f32)      # one bank
                    for e4 in range(4):
                        e = q * 4 + e4
                        nc.tensor.transpose(
                            ytp[:, e4 * 128:(e4 + 1) * 128],
                            ypf[:, e * C:(e + 1) * C],
                            identt[:, :])
                    yts = yts_pool.tile([C, 512], bf16)
                    nc.scalar.activation(yts[:, :], ytp[:, :], COPY,
                                         bias=0.0, scale=1.0)
                    for e4 in range(4):
                        e = q * 4 + e4
                        ex = grp * 16 + e
                        ohoff = (ex * 2 + dirn) * SW
                        epsum = eps_pool.tile([128, SW], f32)
                        nc.tensor.matmul(
                            epsum[:, :],
                            yts[:, e4 * 128:(e4 + 1) * 128],
                            oht[:, ohoff:ohoff + SW],
                            start=True, stop=True)
                        nc.scalar.activation(
                            ebf_k[:, ex * SW:(ex + 1) * SW], epsum[:, :],
                            COPY, bias=float(G * EPS), scale=float(G))
            ed = dramp.tile([128, bn * SW], bf16, tag=f"ed{k}")
            edram[k] = ed
            # (t-part, ex*SW contiguous) -> contiguous 16.9KB per partition row
            nc.gpsimd.dma_start(out=ed[:, :], in_=ebf_k[:, :])

        # ---- phase B: reshuffle DRAM -> (example|dir partition, tau) chunks ----
        ec_tiles = []
        for j in range(nchk):
            ec = ec_pool.tile([128, chk * SW], bf16)
            ec3 = ec[:, :].rearrange("p (t s) -> p t s", s=SW)
            kf = j // 4
            tl0 = (j % 4) * chk
            kb = nblk - 1 - kf
            tb0 = 127 - (j % 4) * chk
            fsrc = edram[kf][:, :].rearrange("t (e s) -> t e s", s=SW)
            bsrc = edram[kb][:, :].rearrange("t (e s) -> t e s", s=SW)
            bslice = slice(tb0, None, -1) if tb0 - chk < 0 else slice(tb0, tb0 - chk, -1)
            nc.sync.dma_start(
                out=ec3[0:64, :, :],
                in_=fsrc[tl0:tl0 + chk, :, :].rearrange("t e s -> e t s"))
            nc.gpsimd.dma_start(
                out=ec3[64:128, :, :],
                in_=bsrc[bslice, :, :].rearrange("t e s -> e t s"))
            ec_tiles.append(ec)

        # ---- phase C: the DP (mirrored state layout, guards at top) ----
        nc.vector.tensor_mul(alpha[:, 0:SW], ec_tiles[0][:, 0:SW], mit[:, :])
        nr = 0
        for tau in range(1, th):
            ec = ec_tiles[tau // chk]
            off = (tau % chk) * SW
            nc.vector.tensor_add(ut[:, :], alpha[:, 0:SW], alpha[:, 1:1 + SW])
            nc.vector.tensor_mul(vt[:, :], alpha[:, 2:2 + SW], m2t[:, :])
            nc.vector.tensor_add(wt[:, :], ut[:, :], vt[:, :])
            nc.vector.tensor_mul(alpha[:, 0:SW], wt[:, :], ec[:, off:off + SW])
            if tau % RESC == 0 and nr < nresc:
                # max over mirrored real states i in [2, 132) (incl. one pad col)
                nc.vector.reduce_max(histt[:, nr:nr + 1], alpha[:, 2:SW], axis=AX)
                nc.vector.reciprocal_approx_fast(sclt[:, :], histt[:, nr:nr + 1])
                nc.vector.tensor_scalar(alpha[:, 0:SW], alpha[:, 0:SW],
                                        sclt[:, :], float(2.0 ** SETPOINT_LOG2),
                                        MULT, MULT)
                nr += 1

        # ---- final: gamma on bwd rows, dump states (host does f64 splice) ----
        nc.vector.tensor_add(ut[:, :], alpha[:, 0:SW], alpha[:, 1:1 + SW])
        nc.vector.tensor_mul(vt[:, :], alpha[:, 2:2 + SW], m2t[:, :])
        nc.vector.tensor_add(wt[:, :], ut[:, :], vt[:, :])
        nc.sync.dma_start(out=a_out.ap(), in_=alpha[:, :])
        nc.sync.dma_start(out=g_out.ap(), in_=wt[:, :])
        nc.sync.dma_start(out=h_out.ap(), in_=histt[:, :])

    nc.compile()
    return nc


def kernel(y_true, y_pred):
    global _BUILT, _LAST_EXEC_NS, _LAST_RES
    from concourse.bass_utils import run_bass_kernel_spmd

    y_true = np.asarray(y_true)
    y_pred = np.ascontiguousarray(np.asarray(y_pred, dtype=np.float32))

    m2f, m2b, mif, mib, clsf, clsb = _host_metadata(y_true)

    if _BUILT is None:
        _BUILT = _build()
    nc = _BUILT

    import ml_dtypes
    bf = ml_dtypes.bfloat16
    ident = np.eye(128, dtype=np.float32)
    in_maps = []
    for c in range(NCORES):
        sl = slice(c * BN, (c + 1) * BN)
        # mirrored layout: reverse the free (state) dim
        m2 = np.concatenate([m2f[sl], m2b[sl]], axis=0)[:, ::-1].astype(bf)
        mi = np.concatenate([mif[sl], mib[sl]], axis=0)[:, ::-1].astype(bf)
        oh = np.zeros((C, BN * 2 * SW), bf)
        for e in range(BN):
            b = c * BN + e
            for dirn, cls in ((0, clsf[b]), (1, clsb[b])):
                colbase = (e * 2 + dirn) * SW
                rcls = cls[::-1]                    # mirrored
                for i in range(SW):
                    if rcls[i] >= 0:
                        oh[rcls[i], colbase + i] = bf(1.0)
        in_maps.append({
            "ypred": y_pred[sl],
            "m2": np.ascontiguousarray(m2),
            "minit": np.ascontiguousarray(mi),
            "onehot": oh,
            "ident": ident,
        })

    import os
    trace = os.environ.get("CTC_TRACE", "") == "1"
    res = run_bass_kernel_spmd(nc, in_maps, list(range(NCORES)), trace=trace)
    _LAST_EXEC_NS = res.exec_time_ns
    _LAST_RES = res

    losses = np.zeros(B, np.float64)
    lng = np.log(np.float64(G))
    setlog = NRESC * SETPOINT_LOG2 * np.log(2.0)
    for c in range(NCORES):
        afin = res.results[c]["afin"].astype(np.float64)   # (128, SW+2) mirrored
        gfin = res.results[c]["gfin"].astype(np.float64)   # (128, SW) mirrored
        hist = res.results[c]["hist"].astype(np.float64)
        acc = np.log(np.maximum(hist[:, :NRESC], 1e-300)).sum(axis=1)
        afs = afin[:, 0:SW][:, ::-1]             # un-mirror -> natural state order
        gfs = gfin[:, :][:, ::-1]
        af = afs[0:64, 0:S]                      # alpha_{T/2-1}[s]
        gm = gfs[64:128, 0:S][:, ::-1]           # W_{T/2-1}[s], u -> s
        P = (af * gm).sum(axis=1)
        lnP = np.log(np.maximum(P, 1e-300))
        losses[c * BN:(c + 1) * BN] = -(
            lnP + acc[:64] + acc[64:128] - 2 * setlog - T * lng)
    return np.float32(losses.mean())



# revision 29
# speedup vs baseline: 1.2312x; 1.0222x over previous
"""CTC loss kernel for Trainium2 (8 NeuronCores, data-parallel over batch).

Strategy
--------
Per core: 64 examples. The CTC forward DP runs in probability space
(4 tensor ops per time step on DVE) with states in the free dim and
(example, direction) packed into the 128 partitions: rows 0-63 run the
forward DP for t=0..255, rows 64-127 run the suffix (backward) DP in
state-reversed coordinates for t=511..256.  The two halves are spliced
at T/2:  P = sum_s alpha_255[s] * W_255[s].

Emissions E[b,t,s] = g*(y_pred[b,t,ext_b[s]] + eps) are produced on the
TensorEngine: per (example, 128-t block), PE-transpose y_pred to
(class, t), then a one-hot matmul (ypT.T @ onehot_b) gathers all 132
state emissions for 128 t steps in one instruction.  The scalar engine
copies PSUM->SBUF fusing the g scale and g*eps bias plus the f32->bf16
cast.  A DRAM round-trip reshuffles (t-part, s) per example into the
DP's (example-part, tau-major) chunk layout.

The state dim is stored MIRRORED (guards at the top) so that 3 of the
4 DP ops have 4-byte-aligned offset-0 operands and hit the DVE 2x_1P
bf16 perf mode (measured: offset-0 non-inplace ops run 2x).

Numerics: bf16 DP state, per-8-step per-example rescale to a 2^43
setpoint (max history written out, logs added back on host), pad states
get all-zero one-hot columns so they decay.  The final splice spans
~e^-180 for tail examples, far outside f32 range, so the two final
state tiles (34KB each) are DMA'd out and the splice runs on host in
f64 (the per-example log + mean were host work already).
"""

import numpy as np

B, T, C, L = 512, 512, 96, 64
BLANK = C - 1
EPS = 1e-7
S = 2 * L + 1          # 129 states
SW = 132               # padded state width (multiple of 4)
NCORES = 8
BN = B // NCORES       # 64 examples per core
TH = T // 2            # split point
RESC = 32              # rescale period
NRESC = (TH - 1) // RESC  # 7 rescales (tau = 32,64,...,224)
SETPOINT_LOG2 = 24     # rescale setpoint 2^24 (headroom for 32 unrescaled steps)
G = 60.646622          # exp(mean_loss/T) boost; keeps alpha ~O(1) per step

_BUILT = None
_LAST_EXEC_NS = None
_LAST_RES = None


def _host_metadata(y_true):
    """ext labels, skip masks, init masks, per-state classes — from y_true.

    Everything is built in natural state order (validated layout), then
    reversed along the free dim at pack time for the mirrored device layout.
    """
    y_true = np.asarray(y_true, dtype=np.int32)
    lbl_len = (y_true != -1).sum(axis=-1).astype(np.int32)
    labels = np.where(y_true != -1, y_true, 0).astype(np.int32)
    ext = np.full((B, S), BLANK, np.int32)
    ext[:, 1::2] = labels
    ext_m2 = np.pad(ext[:, :-2], ((0, 0), (2, 0)), constant_values=BLANK)
    can_skip = ((ext != BLANK) & (ext != ext_m2)).astype(np.float32)

    m2f = np.zeros((B, SW), np.float32)
    m2f[:, :S] = can_skip
    m2b = np.zeros((B, SW), np.float32)
    for u in range(2, S):
        m2b[:, u] = can_skip[:, S - 1 - u + 2]

    mif = np.zeros((B, SW), np.float32)
    mif[:, 0] = 1.0
    mif[:, 1] = 1.0
    mib = np.zeros((B, SW), np.float32)
    mib[np.arange(B), S - 1 - 2 * lbl_len] = 1.0
    mib[np.arange(B), S - 1 - (2 * lbl_len - 1)] = 1.0

    clsf = np.full((B, SW), -1, np.int32)           # -1 -> all-zero column
    clsf[:, :S] = ext
    clsb = np.full((B, SW), -1, np.int32)
    clsb[:, :S] = ext[:, ::-1]
    return m2f, m2b, mif, mib, clsf, clsb


def _build(num_cores=NCORES, t_full=T, bn=BN):
    """Build and schedule the Bass module once."""
    import concourse.bacc as bacc
    import concourse.mybir as mybir
    import concourse.tile as tile
    from contextlib import ExitStack
    from concourse.vector_clock import ScopedClock

    # this walrus build allows a single sem wait per Drain: split the
    # TileContext end-drain's waits across a chain of drains.
    def _patched_drain_and_barrier(self, tick_clock, wait_clock):
        nc = self.nc
        drain_inst = nc.sync.drain()
        wait_clock.add_sem_waits(
            drain_inst.ins, ScopedClock({None: tick_clock.global_clock})
        )
        si = drain_inst.ins.sync_info
        waits = list(si.on_wait) if si and si.on_wait else []
        if len(waits) > 1:
            si.on_wait = waits[:1]
            for w in waits[1:]:
                extra = nc.sync.drain()
                esi = extra.ins.sync_info
                if esi is None:
                    extra.ins.sync_info = mybir.SyncInfo(on_wait=[w], on_update=[])
                else:
                    esi.on_wait = (esi.on_wait or []) + [w]
        nc.all_engine_barrier()
        assert self.sems is not None
        popped = nc._tile_sem_poison_stack.pop()
        assert popped is self._sem_poison
        nc.clear_and_free_semaphores(list(self.sems.allocated().values()))
        nc.all_engine_barrier()

    tile.TileContext._drain_and_barrier = _patched_drain_and_barrier

    f32 = mybir.dt.float32
    bf16 = mybir.dt.bfloat16
    AX = mybir.AxisListType.X
    COPY = mybir.ActivationFunctionType.Copy
    MULT = mybir.AluOpType.mult

    th = t_full // 2
    nblk = t_full // 128
    chk = 32
    nchk = th // chk
    nresc = (th - 1) // RESC

    nc = bacc.Bacc("TRN2", target_bir_lowering=False, debug=False,
                   num_devices=num_cores)
    ypred = nc.dram_tensor("ypred", [bn, t_full, C], f32, kind="ExternalInput")
    m2_in = nc.dram_tensor("m2", [128, SW], bf16, kind="ExternalInput")
    mi_in = nc.dram_tensor("minit", [128, SW], bf16, kind="ExternalInput")
    oh_in = nc.dram_tensor("onehot", [C, bn * 2 * SW], bf16, kind="ExternalInput")
    id_in = nc.dram_tensor("ident", [128, 128], f32, kind="ExternalInput")
    h_out = nc.dram_tensor("hist", [128, max(nresc, 1)], f32, kind="ExternalOutput")
    a_out = nc.dram_tensor("afin", [128, SW + 2], bf16, kind="ExternalOutput")
    g_out = nc.dram_tensor("gfin", [128, SW], bf16, kind="ExternalOutput")

    with tile.TileContext(nc) as tc, ExitStack() as ctx:
        const = ctx.enter_context(tc.tile_pool(name="const", bufs=1))
        dramp = ctx.enter_context(tc.tile_pool(name="edram", bufs=1, space="DRAM"))
        ypf_pool = ctx.enter_context(tc.tile_pool(name="ypf", bufs=3))
        ebf_pool = ctx.enter_context(tc.tile_pool(name="ebf", bufs=3))
        ec_pool = ctx.enter_context(tc.tile_pool(name="ec", bufs=4))
        ytp_pool = ctx.enter_context(tc.tile_pool(name="ytp", bufs=2, space="PSUM"))
        eps_pool = ctx.enter_context(tc.tile_pool(name="eps", bufs=6, space="PSUM"))
        yts_pool = ctx.enter_context(tc.tile_pool(name="yts", bufs=3))

        m2t = const.tile([128, SW], bf16, tag="m2t")
        mit = const.tile([128, SW], bf16, tag="mit")
        oht = const.tile([C, bn * 2 * SW], bf16, tag="oht")
        identt = const.tile([128, 128], f32, tag="identt")
        alpha = const.tile([128, SW + 2], bf16, tag="alpha")
        ut = const.tile([128, SW], bf16, tag="ut")
        vt = const.tile([128, SW], bf16, tag="vt")
        wt = const.tile([128, SW], bf16, tag="wt")
        histt = const.tile([128, max(nresc, 1)], f32, tag="histt")
        sclt = const.tile([128, 1], f32, tag="sclt")

        blk_order = []
        for i in range(nblk // 2):
            blk_order += [i, nblk - 1 - i]

        def _load_ypf(k, grp):
            ypf = ypf_pool.tile([128, 16 * C], f32)
            yp3 = ypf[:, :].rearrange("p (e c) -> p e c", c=C)
            src = ypred.ap()[grp * 16:(grp + 1) * 16,
                             k * 128:(k + 1) * 128, :]
            nc.sync.dma_start(out=yp3[:, :, :],
                              in_=src.rearrange("e t c -> t e c"))
            return ypf

        # first y_pred loads + identity BEFORE the bulky const DMAs so the
        # PE pipeline starts immediately
        prefetched = {(blk_order[0], 0): _load_ypf(blk_order[0], 0)}
        nc.sync.dma_start(out=identt[:, :], in_=id_in.ap())
        prefetched[(blk_order[0], 1)] = _load_ypf(blk_order[0], 1)
        nc.sync.dma_start(out=m2t[:, :], in_=m2_in.ap())
        nc.sync.dma_start(out=mit[:, :], in_=mi_in.ap())
        gw = 16 * 2 * SW
        for g in range(bn // 16):
            nc.sync.dma_start(out=oht[:, g * gw:(g + 1) * gw],
                              in_=oh_in.ap()[:, g * gw:(g + 1) * gw])
        nc.vector.memset(histt[:, :], 0.0)
        nc.vector.memset(alpha[:, :], 0.0)

        # ---- phase A: emissions per 128-t block via PE one-hot matmul ----
        edram = {}
        for k in blk_order:
            ebf_k = ebf_pool.tile([128, bn * SW], bf16)
            dirn = 0 if k < nblk // 2 else 1
            for grp in range(bn // 16):
                ypf = prefetched.pop((k, grp), None)
                if ypf is None:
                    ypf = _load_ypf(k, grp)
                for q in range(4):          # 4 examples per PSUM group
                    ytp = ytp_pool.tile([C, 512], f32)      # one bank
                    for e4 in range(4):
                        e = q * 4 + e4
                        nc.tensor.transpose(
                            ytp[:, e4 * 128:(e4 + 1) * 128],
                            ypf[:, e * C:(e + 1) * C],
                            identt[:, :])
                    yts = yts_pool.tile([C, 512], bf16)
                    nc.scalar.activation(yts[:, :], ytp[:, :], COPY,
                                         bias=0.0, scale=1.0)
                    for e4 in range(4):
                        e = q * 4 + e4
                        ex = grp * 16 + e
                        ohoff = (ex * 2 + dirn) * SW
                        epsum = eps_pool.tile([128, SW], f32)
                        nc.tensor.matmul(
                            epsum[:, :],
                            yts[:, e4 * 128:(e4 + 1) * 128],
                            oht[:, ohoff:ohoff + SW],
                            start=True, stop=True)
                        nc.scalar.activation(
                            ebf_k[:, ex * SW:(ex + 1) * SW], epsum[:, :],
                            COPY, bias=float(G * EPS), scale=float(G))
            ed = dramp.tile([128, bn * SW], bf16, tag=f"ed{k}")
            edram[k] = ed
            # (t-part, ex*SW contiguous) -> contiguous 16.9KB per partition row
            nc.gpsimd.dma_start(out=ed[:, :], in_=ebf_k[:, :])

        # ---- phase B: reshuffle DRAM -> (example|dir partition, tau) chunks ----
        ec_tiles = []
        for j in range(nchk):
            ec = ec_pool.tile([128, chk * SW], bf16)
            ec3 = ec[:, :].rearrange("p (t s) -> p t s", s=SW)
            kf = j // 4
            tl0 = (j % 4) * chk
            kb = nblk - 1 - kf
            tb0 = 127 - (j % 4) * chk
            fsrc = edram[kf][:, :].rearrange("t (e s) -> t e s", s=SW)
            bsrc = edram[kb][:, :].rearrange("t (e s) -> t e s", s=SW)
            bslice = slice(tb0, None, -1) if tb0 - chk < 0 else slice(tb0, tb0 - chk, -1)
            nc.sync.dma_start(
                out=ec3[0:64, :, :],
                in_=fsrc[tl0:tl0 + chk, :, :].rearrange("t e s -> e t s"))
            nc.gpsimd.dma_start(
                out=ec3[64:128, :, :],
                in_=bsrc[bslice, :, :].rearrange("t e s -> e t s"))
            ec_tiles.append(ec)

        # ---- phase C: the DP (mirrored state layout, guards at top) ----
        nc.vector.tensor_mul(alpha[:, 0:SW], ec_tiles[0][:, 0:SW], mit[:, :])
        nr = 0
        for tau in range(1, th):
            ec = ec_tiles[tau // chk]
            off = (tau % chk) * SW
            nc.vector.tensor_add(ut[:, :], alpha[:, 0:SW], alpha[:, 1:1 + SW])
            nc.vector.tensor_mul(vt[:, :], alpha[:, 2:2 + SW], m2t[:, :])
            nc.vector.tensor_add(wt[:, :], ut[:, :], vt[:, :])
            nc.vector.tensor_mul(alpha[:, 0:SW], wt[:, :], ec[:, off:off + SW])
            if tau % RESC == 0 and nr < nresc:
                # max over mirrored real states i in [2, 132) (incl. one pad col)
                nc.vector.reduce_max(histt[:, nr:nr + 1], alpha[:, 2:SW], axis=AX)
                nc.vector.reciprocal_approx_fast(sclt[:, :], histt[:, nr:nr + 1])
                nc.vector.tensor_scalar(alpha[:, 0:SW], alpha[:, 0:SW],
                                        sclt[:, :], float(2.0 ** SETPOINT_LOG2),
                                        MULT, MULT)
                nr += 1

        # ---- final: gamma on bwd rows, dump states (host does f64 splice) ----
        nc.vector.tensor_add(ut[:, :], alpha[:, 0:SW], alpha[:, 1:1 + SW])
        nc.vector.tensor_mul(vt[:, :], alpha[:, 2:2 + SW], m2t[:, :])
        nc.vector.tensor_add(wt[:, :], ut[:, :], vt[:, :])
        nc.sync.dma_start(out=a_out.ap(), in_=alpha[:, :])
        nc.sync.dma_start(out=g_out.ap(), in_=wt[:, :])
        nc.sync.dma_start(out=h_out.ap(), in_=histt[:, :])

    nc.compile()
    return nc


def kernel(y_true, y_pred):
    global _BUILT, _LAST_EXEC_NS, _LAST_RES
    from concourse.bass_utils import run_bass_kernel_spmd

    y_true = np.asarray(y_true)
    y_pred = np.ascontiguousarray(np.asarray(y_pred, dtype=np.float32))

    m2f, m2b, mif, mib, clsf, clsb = _host_metadata(y_true)

    if _BUILT is None:
        _BUILT = _build()
    nc = _BUILT

    import ml_dtypes
    bf = ml_dtypes.bfloat16
    ident = np.eye(128, dtype=np.float32)
    in_maps = []
    for c in range(NCORES):
        sl = slice(c * BN, (c + 1) * BN)
        # mirrored layout: reverse the free (state) dim
        m2 = np.concatenate([m2f[sl], m2b[sl]], axis=0)[:, ::-1].astype(bf)
        mi = np.concatenate([mif[sl], mib[sl]], axis=0)[:, ::-1].astype(bf)
        oh = np.zeros((C, BN * 2 * SW), bf)
        for e in range(BN):
            b = c * BN + e
            for dirn, cls in ((0, clsf[b]), (1, clsb[b])):
                colbase = (e * 2 + dirn) * SW
                rcls = cls[::-1]                    # mirrored
                for i in range(SW):
                    if rcls[i] >= 0:
                        oh[rcls[i], colbase + i] = bf(1.0)
        in_maps.append({
            "ypred": y_pred[sl],
            "m2": np.ascontiguousarray(m2),
            "minit": np.ascontiguousarray(mi),
            "onehot": oh,
            "ident": ident,
        })

    import os
    trace = os.environ.get("CTC_TRACE", "") == "1"
    res = run_bass_kernel_spmd(nc, in_maps, list(range(NCORES)), trace=trace)
    _LAST_EXEC_NS = res.exec_time_ns
    _LAST_RES = res

    losses = np.zeros(B, np.float64)
    lng = np.log(np.float64(G))
    setlog = NRESC * SETPOINT_LOG2 * np.log(2.0)
    for c in range(NCORES):
        afin = res.results[c]["afin"].astype(np.float64)   # (128, SW+2) mirrored
        gfin = res.results[c]["gfin"].astype(np.float64)   # (128, SW) mirrored
        hist = res.results[c]["hist"].astype(np.float64)
        acc = np.log(np.maximum(hist[:, :NRESC], 1e-300)).sum(axis=1)
        afs = afin[:, 0:SW][:, ::-1]             # un-mirror -> natural state order
        gfs = gfin[:, :][:, ::-1]
        af = afs[0:64, 0:S]                      # alpha_{T/2-1}[s]
        gm = gfs[64:128, 0:S][:, ::-1]           # W_{T/2-1}[s], u -> s
        P = (af * gm).sum(axis=1)
        lnP = np.log(np.maximum(P, 1e-300))
        losses[c * BN:(c + 1) * BN] = -(
            lnP + acc[:64] + acc[64:128] - 2 * setlog - T * lng)
    return np.float32(losses.mean())

